# revision 24
# baseline (speedup 1.0000x reference)
"""DiffPool forward on 8 Trainium2 NeuronCores, data-parallel over batch.

B=16 graphs -> 2 per core; identical Bass program per core; host folds params,
shards inputs, combines device-computed loss partial sums.

Masked GAT softmax uses
  exp(leaky_relu(s_j + d_i)) = max(exp(s_j)exp(d_i), exp(.2 s_j)exp(.2 d_i));
factoring exp(d_i) (cancels against the softmax row sum) leaves
  F[j,i] = exp(s_j) * max(1, q_j g_i) * mask[j,i],
  q_j = exp(-.8 s_j), g_i = exp(-.8 d_i)
so no transcendental touches an [N,N] tile.

link_loss: sum((adj - s s^T)^2) = sum(adj) - 2 tr(s^T adj s) + |s^T s|_F^2.
"""

import sys

sys.path.insert(0, '/opt/trn_rl_repo')

import numpy as np

import concourse.bacc as bacc
import concourse.mybir as mybir
from concourse import tile
from concourse.bass import AP
from concourse.bass_utils import run_bass_kernel_spmd

F = mybir.ActivationFunctionType
OP = mybir.AluOpType
AX = mybir.AxisListType
F32 = mybir.dt.float32
F32R = mybir.dt.float32r

B, N, F_IN, HID, OUT, K, H = 16, 1024, 64, 128, 8, 5, 2
BN_EPS = 1e-5
NCORES = 8
G = B // NCORES
NC = N // 128
GK = G * K
GKP = 64
K6 = 6

_cache = {}


def build_program():
    if 'nc' in _cache:
        return _cache['nc']
    nc = bacc.Bacc(None, target_bir_lowering=False, debug=False)

    def din(name, shape, dt=F32):
        return nc.dram_tensor(name, shape, dt, kind="ExternalInput").ap()

    def dout(name, shape, dt=F32):
        return nc.dram_tensor(name, shape, dt, kind="ExternalOutput").ap()

    x_in = din("x", [G, N, F_IN], F32R)
    adj_in = din("adj", [G, N, N], F32R)
    eye_in = din("eye128", [128, 128], F32R)
    eyef_in = din("eye128f", [128, 128])
    eps_in = din("epscol", [128, 1])
    w1_in = din("wext1", [F_IN, 388], F32R)
    w2_in = din("wext2", [2 * HID, 256], F32R)
    wc1_in = din("wc1ext", [HID, 260], F32R)
    wc2_in = din("wc2ext", [2 * HID, 256], F32R)
    p2w_in = din("p2w", [HID, K6], F32R)
    fc1_in = din("fc1", [HID, HID], F32R)
    fc2_in = din("fc2", [HID, OUT], F32R)
    sel_in = din("sel10", [GKP, G], F32R)
    mz_in = din("mzones", [128, 2], F32R)
    ones_in = din("ones128", [128, 1])
    t2g1T_in = din("t2g1T", [2 * HID, N])
    ig1e_in = din("ig1e", [128, NC])
    g2bb_in = din("g2bb", [128, HID])
    cb2e_in = din("cb2e", [128, NC])
    ig2e_in = din("ig2e", [128, NC])
    t2p1T_in = din("t2p1T", [HID, N])
    ig1p_in = din("ig1p", [128, NC])
    t2p2_in = din("t2p2", [N, K6])
    ig2p_in = din("ig2p", [128, NC])
    t2c1_in = din("t2c1", [GKP, 2 * HID])
    ig1c_in = din("ig1c", [GKP, 1])
    t2c2_in = din("t2c2", [GKP, HID])
    ig2c_in = din("ig2c", [GKP, 1])
    fc1b_in = din("fc1b", [HID, 1])
    fc2b_in = din("fc2b", [OUT, 1])
    eyec_in = din("eyec", [GKP, GKP])
    inveyec_in = din("inveyec", [GKP, GKP])
    zk6_in = din("zk6", [128, NC, K6], F32R)
    z64_in = din("zeros64", [GKP, HID], F32R)

    zl_out = dout("z_local", [G, N, HID], F32R)
    zm_out = dout("z_meso", [G, K, HID], F32R)
    s_out = dout("s", [G, N, K])
    ol_out = dout("out_local", [G, OUT])
    om_out = dout("out_meso", [G, OUT])
    st_out = dout("stats", [16, 1])

    with tile.TileContext(nc) as tc:
        with (
            tc.tile_pool(name="const", bufs=1) as cpool,
            tc.tile_pool(name="madj", bufs=1) as mpool,
            tc.tile_pool(name="pers", bufs=1) as ppool,
            tc.tile_pool(name="row", bufs=1) as rpool,
            tc.tile_pool(name="chunk", bufs=2) as kpool,
            tc.tile_pool(name="small", bufs=1) as spool,
            tc.tile_pool(name="pA", bufs=2, space="PSUM") as psA,
            tc.tile_pool(name="pT", bufs=1, space="PSUM") as psT,
            tc.tile_pool(name="pR", bufs=1, space="PSUM") as psR,
        ):
            DMA = nc.sync.dma_start

            def lc(ap_in, shape, dt=F32, tag=None):
                t = cpool.tile(shape, dt, tag=tag)
                DMA(t[:], ap_in[:])
                return t

            eye = lc(eye_in, [128, 128], F32R, "eye")
            eyef = lc(eyef_in, [128, 128], F32, "eyef")
            epscol = lc(eps_in, [128, 1], F32, "epscol")
            w1 = lc(w1_in, [F_IN, 388], F32R, "w1")
            w2 = cpool.tile([128, 2, 256], F32R, tag="w2")
            for h in range(H):
                DMA(w2[:, h, :], w2_in[h * 128:(h + 1) * 128, :])
            wc1 = lc(wc1_in, [HID, 260], F32R, "wc1")
            wc2 = cpool.tile([128, 2, 256], F32R, tag="wc2")
            for h in range(H):
                DMA(wc2[:, h, :], wc2_in[h * 128:(h + 1) * 128, :])
            p2w = lc(p2w_in, [HID, K6], F32R, "p2w")
            fc1 = lc(fc1_in, [HID, HID], F32R, "fc1")
            fc2 = lc(fc2_in, [HID, OUT], F32R, "fc2")
            sel = lc(sel_in, [GKP, G], F32R, "sel")
            mzones = lc(mz_in, [128, 2], F32R, "mz")
            ones = lc(ones_in, [128, 1], F32, "ones")
            t2g1T = cpool.tile([128, 2, N], F32, tag="t2g1T")
            for h in range(H):
                DMA(t2g1T[:, h, :], t2g1T_in[h * 128:(h + 1) * 128, :])
            ig1e = lc(ig1e_in, [128, NC], F32, "ig1e")
            g2bb = lc(g2bb_in, [128, HID], F32, "g2bb")
            cb2e = lc(cb2e_in, [128, NC], F32, "cb2e")
            ig2e = lc(ig2e_in, [128, NC], F32, "ig2e")
            t2p1T = lc(t2p1T_in, [HID, N], F32, "t2p1T")
            ig1p = lc(ig1p_in, [128, NC], F32, "ig1p")
            t2p2 = cpool.tile([128, NC, K6], F32, tag="t2p2")
            for c in range(NC):
                DMA(t2p2[:, c, :], t2p2_in[c * 128:(c + 1) * 128, :])
            ig2p = lc(ig2p_in, [128, NC], F32, "ig2p")
            t2c1 = lc(t2c1_in, [GKP, 2 * HID], F32, "t2c1")
            ig1c = lc(ig1c_in, [GKP, 1], F32, "ig1c")
            t2c2 = lc(t2c2_in, [GKP, HID], F32, "t2c2")
            ig2c = lc(ig2c_in, [GKP, 1], F32, "ig2c")
            fc1b = lc(fc1b_in, [HID, 1], F32, "fc1b")
            fc2b = lc(fc2b_in, [OUT, 1], F32, "fc2b")
            eyec = lc(eyec_in, [GKP, GKP], F32, "eyec")
            inveyec = lc(inveyec_in, [GKP, GKP], F32, "inveyec")

            stats = cpool.tile([128, 16], F32, tag="stats")
            nc.gpsimd.memset(stats[:], 0.0)
            xc10 = cpool.tile([GKP, HID], F32R, tag="xc10")
            DMA(xc10[:], z64_in[:])
            adjc = cpool.tile([GKP, GKP], F32, tag="adjc")
            nc.gpsimd.memset(adjc[:], 0.0)
            mz_all = cpool.tile([HID, G], F32R, tag="mz_all")

            def to_row(wt, nch, tag):
                """wt [128, nch] chunked column vector -> row tile [1, nch*128]
                in node order n = c*128 + p."""
                tp = psT.tile([nch, 128], F32R, tag="pTt")
                nc.tensor.transpose(tp[:], wt[:], eye[:])
                wtT = spool.tile([nch, 128], F32R, tag=f"{tag}T")
                nc.scalar.copy(wtT[:], tp[:])
                row = spool.tile([1, nch * 128], F32R, tag=f"{tag}R")
                a = wtT[:]
                dst = row[:]
                DMA(AP(dst.tensor, dst.offset, [[nch * 128, 1], [1, nch * 128]]),
                    AP(a.tensor, a.offset, [[128, nch], [1, 128]]))
                return row

            def bcast(row_ap, width, tag):
                """row_ap [1, width] -> [128, width] via gpsimd."""
                out = rpool.tile([128, width], row_ap.dtype, tag=tag)
                nc.gpsimd.partition_broadcast(out[:], row_ap, channels=128)
                return out

            def attention(hhat, es, q, grow, m_tiles, nheads, cdim):
                esr = kpool.tile(list(es.shape), F32R, tag="esr")
                nc.vector.tensor_copy(esr[:], es[:])
                """returns per head (np0, np1, rec_cols [128, NC])."""
                res = []
                for h in range(nheads):
                    dg = bcast(grow[h][0:1, :], N, "dg")
                    np0 = psA.tile([cdim, 512], F32, tag="pAa")
                    np1 = psA.tile([cdim, 512], F32, tag="pAb")
                    rp0 = psR.tile([1, 512], F32, tag="pRa")
                    rp1 = psR.tile([1, 512], F32, tag="pRb")
                    for j in range(NC):
                        zt = kpool.tile([128, N], F32, tag="zt")
                        nc.vector.tensor_scalar(
                            zt[:], dg[:], q[:, j, h:h + 1], 1.0, OP.mult, OP.max)
                        ft = kpool.tile([128, N], F32R, tag="ft")
                        eng = nc.gpsimd if j % 4 == 3 else nc.vector
                        eng.tensor_tensor(ft[:], zt[:], m_tiles[j][:], OP.mult)
                        lhs = hhat[:, j, h * cdim:(h + 1) * cdim]
                        st, sp = (j == 0), (j == NC - 1)
                        nc.tensor.matmul(np0[:], lhs, ft[:, 0:512], start=st,
                                         stop=sp)
                        nc.tensor.matmul(np1[:], lhs, ft[:, 512:N], start=st,
                                         stop=sp)
                        nc.tensor.matmul(rp0[:], esr[:, j, h:h + 1],
                                         ft[:, 0:512], start=st, stop=sp)
                        nc.tensor.matmul(rp1[:], esr[:, j, h:h + 1],
                                         ft[:, 512:N], start=st, stop=sp)
                    rrow = spool.tile([1, N], F32, tag="rrow")
                    nc.scalar.copy(rrow[:, 0:512], rp0[:])
                    nc.vector.tensor_copy(rrow[:, 512:N], rp1[:])
                    rcp = psT.tile([128, NC], F32, tag="pTu")
                    for c in range(NC):
                        nc.tensor.transpose(rcp[:, c:c + 1],
                                            rrow[:, c * 128:(c + 1) * 128],
                                            eyef[0:1, 0:1])
                    rcol = spool.tile([128, NC], F32, tag="rcol")
                    nc.vector.tensor_copy(rcol[:], rcp[:])
                    rec = spool.tile([128, NC], F32, tag=f"rec{h}")
                    nc.vector.reciprocal(rec[:], rcol[:])
                    res.append((np0, np1, rec))
                return res

            for g in range(G):
                # ---------- adjacency ----------
                m_tiles = []
                for c in range(NC):
                    mt = mpool.tile([128, N], F32R, tag=f"m{c}")
                    DMA(mt[:], adj_in[g, c * 128:(c + 1) * 128, :])
                    m_tiles.append(mt)
                diag = spool.tile([128, NC], F32R, tag="diag")
                for c in range(NC):
                    src = AP(adj_in.tensor, g * N * N + c * 128 * (N + 1),
                             [[N + 1, 128], [1, 1]])
                    DMA(diag[:, c:c + 1], src)
                rsum = spool.tile([128, NC], F32, tag="rsum")
                for c in range(NC):
                    nc.vector.tensor_reduce(rsum[:, c:c + 1], m_tiles[c][:],
                                            AX.X, OP.add)
                    db = m_tiles[c][:, c * 128:(c + 1) * 128]
                    nc.vector.tensor_tensor(db, db, eye[:], OP.max)
                deg = spool.tile([128, NC], F32, tag="deg")
                nc.vector.scalar_tensor_tensor(deg[:], rsum[:], 1.0, diag[:],
                                               OP.add, OP.subtract)
                dsq = spool.tile([128, NC], F32, tag="dsq")
                nc.scalar.activation(dsq[:], deg[:], F.Sqrt)
                dcol = spool.tile([128, NC], F32, tag="dcol")
                nc.vector.reciprocal(dcol[:], dsq[:])
                nc.vector.tensor_reduce(stats[:, 0 + g:1 + g], rsum[:], AX.X,
                                        OP.add)

                # ---------- x load + transpose ----------
                xT = ppool.tile([F_IN, N], F32R, tag="xT")
                for c in range(NC):
                    xt = kpool.tile([128, F_IN], F32R, tag="xt")
                    DMA(xt[:], x_in[g, c * 128:(c + 1) * 128, :])
                    xp = psT.tile([F_IN, 128], F32R, tag="pTt")
                    nc.tensor.transpose(xp[:], xt[:], eye[:])
                    nc.scalar.copy(xT[:, c * 128:(c + 1) * 128], xp[:])

                # ---------- h_ext = x @ [W1|as|ad|p1W] ----------
                hhat1 = ppool.tile([128, NC, 2 * HID], F32R, tag="hhat1")
                es1 = ppool.tile([128, NC, H], F32, tag="es1")
                q1 = ppool.tile([128, NC, H], F32, tag="q1")
                dxwp = ppool.tile([128, NC, HID], F32R, tag="dxwp")
                for c in range(NC):
                    hp = psA.tile([128, 388], F32, tag="pAa")
                    nc.tensor.matmul(hp[:], xT[:, c * 128:(c + 1) * 128], w1[:],
                                     start=True, stop=True)
                    nc.scalar.activation(es1[:, c, :], hp[:, 256:258], F.Exp)
                    nc.scalar.activation(q1[:, c, :], hp[:, 256:258], F.Exp,
                                         scale=-0.8)
                    for h in range(H):
                        nc.vector.tensor_scalar(
                            hhat1[:, c, h * HID:(h + 1) * HID],
                            hp[:, h * HID:(h + 1) * HID], es1[:, c, h:h + 1],
                            None, OP.mult)
                    nc.vector.tensor_scalar(dxwp[:, c, :], hp[:, 260:388],
                                            dcol[:, c:c + 1], None, OP.mult)

                g1rows = []
                for h in range(H):
                    adp0 = psR.tile([1, 512], F32, tag="pRa")
                    adp1 = psR.tile([1, 512], F32, tag="pRb")
                    nc.tensor.matmul(adp0[:], w1[:, 258 + h:259 + h],
                                     xT[:, 0:512], start=True, stop=True)
                    nc.tensor.matmul(adp1[:], w1[:, 258 + h:259 + h],
                                     xT[:, 512:N], start=True, stop=True)
                    g1rowh = spool.tile([1, N], F32, tag=f"g1row{h}")
                    nc.scalar.activation(g1rowh[:, 0:512], adp0[:], F.Exp,
                                         scale=-0.8)
                    nc.scalar.activation(g1rowh[:, 512:N], adp1[:], F.Exp,
                                         scale=-0.8)
                    g1rows.append(g1rowh)

                # ---------- GAT1 ----------
                att1 = attention(hhat1, es1, q1, g1rows, m_tiles, H, HID)
                z1T = []
                for h in range(H):
                    np0, np1, rec = att1[h]
                    wt = spool.tile([128, NC], F32R, tag="wt")
                    nc.vector.tensor_tensor(wt[:], rec[:], ig1e[:], OP.mult)
                    wrow = to_row(wt, NC, "w1h")
                    wb = bcast(wrow[:], N, "wbr")
                    zT = ppool.tile([128, N], F32R, tag=f"z1T{h}")
                    for i2 in range(2):
                        sl = slice(i2 * 512, (i2 + 1) * 512)
                        npp = np0 if i2 == 0 else np1
                        u = kpool.tile([128, 512], F32, tag="u1")
                        nc.vector.scalar_tensor_tensor(
                            u[:], npp[:], 1.0, wb[:, sl], OP.mult, OP.mult)
                        v = kpool.tile([128, 512], F32, tag="v1")
                        eng = nc.gpsimd if i2 == 1 else nc.vector
                        eng.tensor_tensor(v[:], u[:],
                                          t2g1T[:, h, sl],
                                          OP.add)
                        nc.scalar.activation(zT[:, sl], v[:], F.Relu)
                    z1T.append(zT)

                # ---------- GAT2 ----------
                hhat2 = ppool.tile([128, NC, HID], F32R, tag="hhat2")
                es2 = ppool.tile([128, NC, 1], F32, tag="es2")
                q2 = ppool.tile([128, NC, 1], F32, tag="q2")
                for c in range(NC):
                    h2p = psA.tile([128, 256], F32, tag="pAb")
                    for h in range(H):
                        nc.tensor.matmul(h2p[:],
                                         z1T[h][:, c * 128:(c + 1) * 128],
                                         w2[:, h, :],
                                         start=(h == 0), stop=(h == 1))
                    nc.scalar.activation(es2[:, c, :], h2p[:, HID:HID + 1],
                                         F.Exp)
                    nc.scalar.activation(q2[:, c, :], h2p[:, HID:HID + 1],
                                         F.Exp, scale=-0.8)
                    nc.vector.tensor_scalar(hhat2[:, c, :], h2p[:, 0:HID],
                                            es2[:, c, 0:1], None, OP.mult)
                ad20 = psR.tile([1, 512], F32, tag="pRa")
                ad21 = psR.tile([1, 512], F32, tag="pRb")
                for h in range(H):
                    st, sp = (h == 0), (h == 1)
                    nc.tensor.matmul(ad20[:],
                                     w2[:, h, HID + 1:HID + 2],
                                     z1T[h][:, 0:512], start=st, stop=sp)
                    nc.tensor.matmul(ad21[:],
                                     w2[:, h, HID + 1:HID + 2],
                                     z1T[h][:, 512:N], start=st, stop=sp)
                g2row = spool.tile([1, N], F32, tag="g2row")
                nc.scalar.activation(g2row[:, 0:512], ad20[:], F.Exp, scale=-0.8)
                nc.scalar.activation(g2row[:, 512:N], ad21[:], F.Exp, scale=-0.8)

                att2 = attention(hhat2, es2, q2, [g2row], m_tiles, 1, HID)
                np0, np1, rec2 = att2[0]
                zl = ppool.tile([128, NC, HID], F32R, tag="zl")
                for c in range(NC):
                    npp = np0 if c < 4 else np1
                    off = (c % 4) * 128
                    tsb = kpool.tile([128, 128], F32R, tag="tsb")
                    nc.scalar.copy(tsb[:], npp[:, off:off + 128])
                    tp = psT.tile([128, 128], F32R, tag="pTt")
                    nc.tensor.transpose(tp[:], tsb[:], eye[:])
                    zraw = kpool.tile([128, HID], F32, tag="zraw")
                    nc.vector.scalar_tensor_tensor(
                        zraw[:], tp[:], rec2[:, c:c + 1],
                        g2bb[:], OP.mult, OP.add)
                    v2 = kpool.tile([128, HID], F32, tag="v2")
                    nc.vector.tensor_scalar(v2[:], zraw[:], 0.0, None, OP.max)
                    nc.scalar.activation(zl[:, c, :], v2[:], F.Relu,
                                         scale=ig2e[:, c:c + 1],
                                         bias=cb2e[:, c:c + 1])
                    DMA(zl_out[g, c * 128:(c + 1) * 128, :], zl[:, c, :])

                # ---------- pool branch GCN1 (flipped) ----------
                gp0 = psA.tile([HID, 512], F32, tag="pAa")
                gp1 = psA.tile([HID, 512], F32, tag="pAb")
                for j in range(NC):
                    st, sp = (j == 0), (j == NC - 1)
                    nc.tensor.matmul(gp0[:], dxwp[:, j, :], m_tiles[j][:, 0:512],
                                     start=st, stop=sp)
                    nc.tensor.matmul(gp1[:], dxwp[:, j, :], m_tiles[j][:, 512:N],
                                     start=st, stop=sp)
                wpt = spool.tile([128, NC], F32R, tag="wpt")
                nc.vector.tensor_tensor(wpt[:], dcol[:], ig1p[:], OP.mult)
                wprow = to_row(wpt, NC, "wp")
                wpb = bcast(wprow[:], N, "wbr")
                s1T = ppool.tile([HID, N], F32R, tag="s1T")
                for i2 in range(2):
                    sl = slice(i2 * 512, (i2 + 1) * 512)
                    npp = gp0 if i2 == 0 else gp1
                    u = kpool.tile([128, 512], F32, tag="u1")
                    nc.vector.scalar_tensor_tensor(
                        u[:], npp[:], 1.0, wpb[:, sl], OP.mult, OP.mult)
                    v = kpool.tile([128, 512], F32, tag="v1")
                    nc.gpsimd.tensor_tensor(v[:], u[:], t2p1T[:, sl], OP.add)
                    nc.scalar.activation(s1T[:, sl], v[:], F.Relu)

                # ---------- GCN2 ----------
                dsw = ppool.tile([128, NC, K6], F32R, tag="dsw")
                for c in range(NC):
                    swp = psT.tile([128, K6], F32, tag="pTt")
                    nc.tensor.matmul(swp[:], s1T[:, c * 128:(c + 1) * 128],
                                     p2w[:], start=True, stop=True)
                    nc.vector.tensor_scalar(dsw[:, c, :], swp[:],
                                            dcol[:, c:c + 1], None, OP.mult)
                sp0 = psA.tile([K6, 512], F32, tag="pAa")
                sp1 = psA.tile([K6, 512], F32, tag="pAb")
                for j in range(NC):
                    st, sp_ = (j == 0), (j == NC - 1)
                    nc.tensor.matmul(sp0[:], dsw[:, j, :], m_tiles[j][:, 0:512],
                                     start=st, stop=sp_)
                    nc.tensor.matmul(sp1[:], dsw[:, j, :], m_tiles[j][:, 512:N],
                                     start=st, stop=sp_)
                s2T = spool.tile([K6, N], F32R, tag="s2T")
                nc.scalar.copy(s2T[:, 0:512], sp0[:])
                nc.vector.tensor_copy(s2T[:, 512:N], sp1[:])
                wp2 = spool.tile([128, NC], F32, tag="wp2")
                nc.vector.tensor_tensor(wp2[:], dcol[:], ig2p[:], OP.mult)
                slog = ppool.tile([128, NC, K6], F32, tag="slog")
                for c in range(NC):
                    tpk = psT.tile([128, K6], F32R, tag="pTt")
                    nc.tensor.transpose(tpk[:], s2T[:, c * 128:(c + 1) * 128],
                                        eye[0:K6, 0:K6])
                    nc.vector.scalar_tensor_tensor(
                        slog[:, c, :], tpk[:], wp2[:, c:c + 1],
                        t2p2[:, c, :], OP.mult, OP.add)

                # ---------- double softmax over k ----------
                s_f = ppool.tile([128, NC, K6], F32R, tag="s_f")
                DMA(s_f[:], zk6_in[:])
                cur = slog[:, :, 0:K]
                for rep in range(2):
                    mx = kpool.tile([128, NC], F32, tag="mx")
                    nc.vector.tensor_reduce(mx[:], cur, AX.X, OP.max,
                                            negate=True)
                    ex = kpool.tile([128, NC, K], F32, tag="ex")
                    nc.vector.scalar_tensor_tensor(
                        ex[:], cur, 1.0, mx[:].broadcast_to([128, NC, K]),
                        OP.mult, OP.add)
                    ex2 = kpool.tile([128, NC, K], F32, tag="ex2")
                    nc.scalar.activation(ex2[:], ex[:], F.Exp)
                    sm = kpool.tile([128, NC], F32, tag="sm")
                    nc.vector.tensor_reduce(sm[:], ex2[:], AX.X, OP.add)
                    rc = kpool.tile([128, NC], F32, tag="rc")
                    nc.vector.reciprocal(rc[:], sm[:])
                    dst = kpool.tile([128, NC, K], F32, tag="smid")
                    nc.vector.scalar_tensor_tensor(
                        dst[:], ex2[:], 1.0, rc[:].broadcast_to([128, NC, K]),
                        OP.mult, OP.mult)
                    if rep == 1:
                        nc.vector.tensor_copy(s_f[:, :, 0:K], dst[:])
                    if rep == 0:
                        for c in range(NC):
                            DMA(s_out[g, c * 128:(c + 1) * 128, :], dst[:, c, :])
                    cur = dst[:]

                # ---------- ent loss ----------
                lg = kpool.tile([128, NC, K], F32, tag="lg")
                nc.scalar.activation(lg[:], s_f[:, :, 0:K], F.Ln,
                                     bias=epscol[:, 0:1])
                lg2 = kpool.tile([128, NC, K], F32, tag="lg2")
                entc = spool.tile([128, 1], F32, tag="entc")
                nc.vector.scalar_tensor_tensor(lg2[:], lg[:], 1.0,
                                               s_f[:, :, 0:K],
                                               OP.mult, OP.mult,
                                               accum_out=entc[:])
                nc.vector.tensor_scalar(stats[:, 2 + g:3 + g], entc[:], -1.0,
                                        None, OP.mult)

                # ---------- diffpool ----------
                ap0 = psA.tile([K6, 512], F32, tag="pAa")
                ap1 = psA.tile([K6, 512], F32, tag="pAb")
                for j in range(NC):
                    st, sp_ = (j == 0), (j == NC - 1)
                    nc.tensor.matmul(ap0[:], s_f[:, j, :],
                                     m_tiles[j][:, 0:512], start=st, stop=sp_)
                    nc.tensor.matmul(ap1[:], s_f[:, j, :],
                                     m_tiles[j][:, 512:N], start=st, stop=sp_)
                sTA = spool.tile([K6, N], F32R, tag="sTA")
                nc.scalar.copy(sTA[:, 0:512], ap0[:])
                nc.vector.tensor_copy(sTA[:, 512:N], ap1[:])
                vAs = ppool.tile([128, NC, K6], F32R, tag="vAs")
                for c in range(NC):
                    tpk = psT.tile([128, K6], F32R, tag="pTt")
                    nc.tensor.transpose(tpk[:], sTA[:, c * 128:(c + 1) * 128],
                                        eye[0:K6, 0:K6])
                    nc.vector.tensor_copy(vAs[:, c, :], tpk[:])
                wdg = spool.tile([128, NC], F32, tag="wdg")
                nc.vector.tensor_scalar(wdg[:], diag[:], -1.0, 1.0, OP.mult,
                                        OP.add)
                sw_ = ppool.tile([128, NC, K6], F32R, tag="sw_")
                for c in range(NC):
                    nc.vector.tensor_scalar(sw_[:, c, :], s_f[:, c, :],
                                            wdg[:, c:c + 1], None, OP.mult)
                oasb = spool.tile([K6, 2 * K6 + HID], F32, tag="oasb")
                corr = spool.tile([K6, K6], F32, tag="corr")
                for grp in range(4):
                    gp = psT.tile([K6, 2 * K6 + HID], F32, tag="pTt")
                    rng_ = [(0, K6), (K6, 2 * K6), (2 * K6, 2 * K6 + HID),
                            (0, K6)][grp]
                    for c in range(NC):
                        st, sp_ = (c == 0), (c == NC - 1)
                        lhs = sw_[:, c, :] if grp == 3 else s_f[:, c, :]
                        rhs = [vAs[:, c, :], s_f[:, c, :], zl[:, c, :],
                               s_f[:, c, :]][grp]
                        nc.tensor.matmul(gp[:, rng_[0]:rng_[1]], lhs, rhs,
                                         start=st, stop=sp_)
                    if grp == 3:
                        nc.scalar.copy(corr[:], gp[:, 0:K6])
                    else:
                        nc.scalar.copy(oasb[:, rng_[0]:rng_[1]],
                                       gp[:, rng_[0]:rng_[1]])
                oadj = spool.tile([K, K], F32, tag="oadj")
                nc.vector.tensor_tensor(oadj[:], oasb[0:K, 0:K],
                                        corr[0:K, 0:K], OP.subtract)
                tr2 = spool.tile([K, K], F32, tag="tr2")
                nc.vector.tensor_tensor(tr2[:], oadj[:], eyec[0:K, 0:K],
                                        OP.mult)
                nc.vector.tensor_reduce(stats[0:K, 4 + g:5 + g], tr2[:], AX.X,
                                        OP.add)
                stsq = spool.tile([K, K], F32, tag="stsq")
                nc.vector.tensor_tensor(stsq[:], oasb[0:K, K6:K6 + K],
                                        oasb[0:K, K6:K6 + K], OP.mult)
                nc.vector.tensor_reduce(stats[0:K, 6 + g:7 + g], stsq[:], AX.X,
                                        OP.add)
                nc.vector.tensor_copy(xc10[g * 32:g * 32 + K, :],
                                      oasb[0:K, 2 * K6:])
                nc.vector.tensor_copy(adjc[g * 32:g * 32 + K,
                                           g * 32:g * 32 + K], oadj[:])

                # ---------- out_local mean ----------
                mzp = psT.tile([HID, 2], F32, tag="pTu")
                for c in range(NC):
                    nc.tensor.matmul(mzp[:], zl[:, c, :], mzones[:],
                                     start=(c == 0), stop=(c == NC - 1))
                nc.scalar.copy(mz_all[:, g:g + 1], mzp[:, 0:1])

            # ================= coarse branch (both graphs, 10 rows) =========
            adjcsl = spool.tile([GKP, GKP], F32, tag="adjcsl")
            nc.vector.tensor_tensor(adjcsl[:], adjc[:], inveyec[:], OP.mult)
            nc.vector.tensor_tensor(adjcsl[:], adjcsl[:], eyec[:], OP.add)
            maskc = spool.tile([GKP, GKP], F32, tag="maskc")
            nc.vector.tensor_scalar(maskc[:], adjcsl[:], 0.0, None,
                                    OP.not_equal)

            def coarse_gat(xin_t, wsl, nheads, cdim, t2, igc, zname):
                nin = xin_t.shape[1]
                nf = nheads * cdim
                wcols = wsl[0].shape[-1]
                nchunks = (nin + 127) // 128
                xtp = []
                for h2 in range(nchunks):
                    w = min(128, nin - h2 * 128)
                    tp = psT.tile([128, GKP], F32R, tag="pTt")
                    nc.tensor.transpose(tp[0:w, :],
                                        xin_t[:, h2 * 128:h2 * 128 + w],
                                        eye[0:GKP, 0:GKP])
                    xt = spool.tile([128, GKP], F32R, tag=f"{zname}xT{h2}")
                    nc.scalar.copy(xt[0:w, :], tp[0:w, :])
                    xtp.append((xt, w))
                hcp = psT.tile([GKP, wcols], F32, tag="pTu")
                for i, (xt, w) in enumerate(xtp):
                    nc.tensor.matmul(hcp[:], xt[0:w, :], wsl[i],
                                     start=(i == 0), stop=(i == nchunks - 1))
                hc = spool.tile([GKP, wcols], F32R, tag=f"{zname}hc")
                nc.scalar.copy(hc[:], hcp[:])
                esc = spool.tile([GKP, nheads], F32, tag=f"{zname}es")
                nc.scalar.activation(esc[:], hc[:, nf:nf + nheads], F.Exp)
                qc = spool.tile([GKP, nheads], F32, tag=f"{zname}q")
                nc.scalar.activation(qc[:], hc[:, nf:nf + nheads], F.Exp,
                                     scale=-0.8)
                gcrs = []
                for h in range(nheads):
                    adps = psT.tile([1, GKP], F32, tag="pTt")
                    for i, (xt, w) in enumerate(xtp):
                        nc.tensor.matmul(
                            adps[:],
                            wsl[i][:, nf + nheads + h:nf + nheads + h + 1],
                            xt[0:w, :], start=(i == 0), stop=(i == nchunks - 1))
                    gcrh = spool.tile([1, GKP], F32, tag=f"{zname}gr{h}")
                    nc.scalar.activation(gcrh[:], adps[:], F.Exp, scale=-0.8)
                    gcrs.append(gcrh)
                zc = spool.tile([GKP, nf], F32R, tag=f"{zname}z")
                for h in range(nheads):
                    hhc = spool.tile([GKP, cdim], F32R, tag=f"{zname}hh")
                    nc.vector.tensor_scalar(hhc[:],
                                            hc[:, h * cdim:(h + 1) * cdim],
                                            esc[:, h:h + 1], None, OP.mult)
                    dgc = spool.tile([GKP, GKP], F32, tag=f"{zname}dg")
                    nc.gpsimd.partition_broadcast(dgc[:], gcrs[h][0:1, :],
                                                  channels=GKP)
                    zcc = spool.tile([GKP, GKP], F32, tag=f"{zname}zc")
                    nc.vector.tensor_scalar(zcc[:], dgc[:], qc[:, h:h + 1], 1.0,
                                            OP.mult, OP.max)
                    fcc = spool.tile([GKP, GKP], F32R, tag=f"{zname}fc")
                    nc.vector.tensor_tensor(fcc[:], zcc[:], maskc[:], OP.mult)
                    nump = psT.tile([cdim, GKP], F32, tag="pTt")
                    nc.tensor.matmul(nump[:], hhc[:], fcc[:], start=True,
                                     stop=True)
                    escr = spool.tile([GKP, 1], F32R, tag=f"{zname}esr")
                    nc.vector.tensor_copy(escr[:], esc[:, h:h + 1])
                    rpc = psT.tile([1, GKP], F32, tag="pTu")
                    nc.tensor.matmul(rpc[:], escr[:], fcc[:], start=True,
                                     stop=True)
                    nsb = spool.tile([cdim, GKP], F32R, tag=f"{zname}nsb")
                    nc.scalar.copy(nsb[:], nump[:])
                    rsb = spool.tile([1, GKP], F32, tag=f"{zname}rsb")
                    nc.scalar.copy(rsb[:], rpc[:])
                    ntp = psT.tile([GKP, cdim], F32R, tag="pTt")
                    nc.tensor.transpose(ntp[:], nsb[:], eye[:])
                    rtp = psT.tile([GKP, 1], F32, tag="pTu")
                    nc.tensor.transpose(rtp[:], rsb[:], eyef[0:1, 0:1])
                    rcc = spool.tile([GKP, 1], F32, tag=f"{zname}rcc")
                    nc.vector.tensor_copy(rcc[:], rtp[:])
                    recc = spool.tile([GKP, 1], F32, tag=f"{zname}recc")
                    nc.vector.reciprocal(recc[:], rcc[:])
                    wcc = spool.tile([GKP, 1], F32, tag=f"{zname}wcc")
                    nc.vector.tensor_tensor(wcc[:], recc[:], igc[:], OP.mult)
                    zpre = spool.tile([GKP, cdim], F32, tag=f"{zname}zpre")
                    nc.vector.scalar_tensor_tensor(
                        zpre[:], ntp[:], wcc[:, 0:1],
                        t2[:, h * cdim:(h + 1) * cdim], OP.mult, OP.add)
                    nc.scalar.activation(zc[:, h * cdim:(h + 1) * cdim],
                                         zpre[:], F.Relu)
                return zc

            z1c = coarse_gat(xc10, [wc1[:]], H, HID, t2c1, ig1c, "c1")
            zmeso = coarse_gat(z1c, [wc2[:, 0, :], wc2[:, 1, :]], 1, HID, t2c2, ig2c, "c2")
            for g in range(G):
                DMA(zm_out[g], zmeso[g * 32:g * 32 + K, :])

            # ---------- readout ----------
            mmp = psT.tile([HID, G], F32, tag="pTt")
            nc.tensor.matmul(mmp[:], zmeso[:], sel[:], start=True, stop=True)
            mmT = spool.tile([HID, G], F32R, tag="mmT")
            nc.scalar.copy(mmT[:], mmp[:])
            omp = psT.tile([HID, G], F32, tag="pTu")
            nc.tensor.matmul(omp[:], fc1[:], mmT[:], start=True, stop=True)
            omT = spool.tile([HID, G], F32R, tag="omT")
            nc.scalar.activation(omT[:], omp[:], F.Relu, bias=fc1b[:, 0:1])
            om2p = psT.tile([OUT, G], F32, tag="pTt")
            nc.tensor.matmul(om2p[:], fc2[:], omT[:], start=True, stop=True)
            omf = spool.tile([OUT, G], F32, tag="omf")
            nc.vector.tensor_scalar(omf[:], om2p[:], fc2b[:, 0:1], None, OP.add)
            olp = psT.tile([OUT, G], F32, tag="pTu")
            nc.tensor.matmul(olp[:], fc2[:], mz_all[:], start=True, stop=True)
            olf = spool.tile([OUT, G], F32, tag="olf")
            nc.vector.tensor_scalar(olf[:], olp[:], fc2b[:, 0:1], None, OP.add)
            aom = om_out[:]
            DMA(AP(aom.tensor, aom.offset, [[1, OUT], [OUT, G]]), omf[:])
            aol = ol_out[:]
            DMA(AP(aol.tensor, aol.offset, [[1, OUT], [OUT, G]]), olf[:])

            # ---------- stats ----------
            stp = psT.tile([16, 1], F32, tag="pTu")
            nc.tensor.matmul(stp[:], stats[:], ones[:], start=True, stop=True)
            stsb = spool.tile([16, 1], F32, tag="stsb")
            nc.scalar.copy(stsb[:], stp[:])
            DMA(st_out[:], stsb[:])

    nc.compile()
    _cache['nc'] = nc
    return nc


def _fold_params(p):
    d = {k: np.asarray(v, np.float64) for k, v in p.items()}

    def bnfold(pre):
        ig = d[pre + '_g'] / np.sqrt(d[pre + '_v'] + BN_EPS)
        c = d[pre + '_b'] - d[pre + '_m'] * ig
        return ig, c

    out = {}
    g1W = d['g1_W']
    was1 = np.stack([g1W.reshape(F_IN, H, HID)[:, h, :] @ d['g1_as'][h]
                     for h in range(H)], 1)
    wad1 = np.stack([g1W.reshape(F_IN, H, HID)[:, h, :] @ d['g1_ad'][h]
                     for h in range(H)], 1)
    out['wext1'] = np.concatenate([g1W, was1, wad1, d['p1_W']], 1)
    was2 = d['g2_W'] @ d['g2_as'][0]
    wad2 = d['g2_W'] @ d['g2_ad'][0]
    out['wext2'] = np.concatenate(
        [d['g2_W'], was2[:, None], wad2[:, None], np.zeros((2 * HID, 126))], 1)
    c1W = d['c1_W']
    wasc = np.stack([c1W.reshape(HID, H, HID)[:, h, :] @ d['c1_as'][h]
                     for h in range(H)], 1)
    wadc = np.stack([c1W.reshape(HID, H, HID)[:, h, :] @ d['c1_ad'][h]
                     for h in range(H)], 1)
    out['wc1ext'] = np.concatenate([c1W, wasc, wadc], 1)
    wasc2 = d['c2_W'] @ d['c2_as'][0]
    wadc2 = d['c2_W'] @ d['c2_ad'][0]
    out['wc2ext'] = np.concatenate(
        [d['c2_W'], wasc2[:, None], wadc2[:, None], np.zeros((2 * HID, 126))],
        1)
    out['p2w'] = np.concatenate([d['p2_W'], np.zeros((HID, 1))], 1)
    out['fc1'] = d['fc1_W']
    out['fc2'] = d['fc2_W']
    out['fc1b'] = d['fc1_b'][:, None]
    out['fc2b'] = d['fc2_b'][:, None]

    ig1e, c1e = bnfold('bn1e')
    ig2e, c2e = bnfold('bn2e')
    ig1p, c1p = bnfold('bn1p')
    ig2p, c2p = bnfold('bn2p')
    ig1c, c1c = bnfold('bn1c')
    ig2c, c2c = bnfold('bn2c')

    def cols(v):
        return np.ascontiguousarray(v.reshape(NC, 128).T)

    out['ig1e'] = cols(ig1e)
    out['ig2e'] = cols(ig2e)
    out['ig1p'] = cols(ig1p)
    out['ig2p'] = cols(ig2p)
    out['t2g1T'] = np.outer(d['g1_b'], ig1e) + c1e[None, :]
    out['g2bb'] = np.tile(d['g2_b'][None, :], (128, 1))
    out['cb2e'] = cols(c2e)
    out['t2p1T'] = np.outer(d['p1_b'], ig1p) + c1p[None, :]
    out['t2p2'] = np.concatenate([np.outer(ig2p, d['p2_b']) + c2p[:, None], np.zeros((N, 1))], 1)
    t2c1 = np.zeros((64, 2 * HID))
    t2c2 = np.zeros((64, HID))
    ig1cp = np.zeros((64, 1))
    ig2cp = np.zeros((64, 1))
    for g in range(G):
        t2c1[g * 32:g * 32 + K] = np.outer(ig1c, d['c1_b']) + c1c[:, None]
        t2c2[g * 32:g * 32 + K] = np.outer(ig2c, d['c2_b']) + c2c[:, None]
        ig1cp[g * 32:g * 32 + K, 0] = ig1c
        ig2cp[g * 32:g * 32 + K, 0] = ig2c
    out['t2c1'] = t2c1
    out['t2c2'] = t2c2
    out['ig1c'] = ig1cp
    out['ig2c'] = ig2cp

    out['eye128'] = np.eye(128)
    out['eye128f'] = np.eye(128)
    out['epscol'] = np.full((128, 1), 1e-15)
    out['zk6'] = np.zeros((128, NC, 6))
    out['zeros64'] = np.zeros((64, HID))
    out['eyec'] = np.eye(64)
    out['inveyec'] = 1.0 - np.eye(64)
    sel = np.zeros((64, G))
    for g in range(G):
        sel[g * 32:g * 32 + K, g] = 1.0 / K
    out['sel10'] = sel
    out['mzones'] = np.concatenate([np.full((128, 1), 1.0 / N), np.zeros((128, 1))], 1)
    out['ones128'] = np.ones((128, 1))
    return {k: np.ascontiguousarray(v, dtype=np.float32)
            for k, v in out.items()}


def kernel(x_dense, adj_dense, params):
    x = np.ascontiguousarray(np.asarray(x_dense), np.float32)
    adj = np.ascontiguousarray(np.asarray(adj_dense), np.float32)
    pf = _fold_params({k: np.asarray(v) for k, v in params.items()})
    nc = build_program()
    in_maps = []
    for core in range(NCORES):
        m = dict(pf)
        m['x'] = x[core * G:(core + 1) * G]
        m['adj'] = adj[core * G:(core + 1) * G]
        in_maps.append(m)
    res = run_bass_kernel_spmd(nc, in_maps, core_ids=list(range(NCORES)))
    z_local = np.concatenate([r['z_local'] for r in res.results], 0)
    z_meso = np.concatenate([r['z_meso'] for r in res.results], 0)
    s = np.concatenate([r['s'] for r in res.results], 0)
    out_local = np.concatenate([r['out_local'] for r in res.results], 0)
    out_meso = np.concatenate([r['out_meso'] for r in res.results], 0)
    link_sq = 0.0
    ent_sum = 0.0
    for r in res.results:
        st = r['stats'][:, 0].astype(np.float64)
        for g in range(G):
            link_sq += st[0 + g] - 2.0 * st[4 + g] + st[6 + g]
            ent_sum += st[2 + g]
    link_loss = np.float32(np.sqrt(link_sq) / (B * N * N))
    ent_loss = np.float32(ent_sum / (B * N))
    return (z_local, z_meso, s, out_local, out_meso, link_loss, ent_loss)


# revision 28
# speedup vs baseline: 4693.2390x; 4693.2390x over previous
"""DiffPool forward on 8 Trainium2 NeuronCores, data-parallel over batch.

B=16 graphs -> 2 per core; identical Bass program per core; host folds params,
shards inputs, combines device-computed loss partial sums.

Masked GAT softmax uses
  exp(leaky_relu(s_j + d_i)) = max(exp(s_j)exp(d_i), exp(.2 s_j)exp(.2 d_i));
factoring exp(d_i) (cancels against the softmax row sum) leaves
  F[j,i] = exp(s_j) * max(1, q_j g_i) * mask[j,i],
  q_j = exp(-.8 s_j), g_i = exp(-.8 d_i)
so no transcendental touches an [N,N] tile.

link_loss: sum((adj - s s^T)^2) = sum(adj) - 2 tr(s^T adj s) + |s^T s|_F^2.
"""

import sys

sys.path.insert(0, '/opt/trn_rl_repo')

import numpy as np

import concourse.bacc as bacc
import concourse.mybir as mybir
from concourse import tile
from concourse.bass import AP
from concourse.bass_utils import run_bass_kernel_spmd

F = mybir.ActivationFunctionType
OP = mybir.AluOpType
AX = mybir.AxisListType
F32 = mybir.dt.float32
F32R = mybir.dt.float32r

B, N, F_IN, HID, OUT, K, H = 16, 1024, 64, 128, 8, 5, 2
BN_EPS = 1e-5
NCORES = 8
G = B // NCORES
NC = N // 128
GK = G * K
GKP = 64
K6 = 6

_cache = {}


def build_program():
    if 'nc' in _cache:
        return _cache['nc']
    nc = bacc.Bacc(None, target_bir_lowering=False, debug=False)

    def din(name, shape, dt=F32):
        return nc.dram_tensor(name, shape, dt, kind="ExternalInput").ap()

    def dout(name, shape, dt=F32):
        return nc.dram_tensor(name, shape, dt, kind="ExternalOutput").ap()

    x_in = din("x", [G, N, F_IN], F32R)
    adj_in = din("adj", [G, N, N], F32R)
    eye_in = din("eye128", [128, 128], F32R)
    eyef_in = din("eye128f", [128, 128])
    eps_in = din("epscol", [128, 1])
    w1_in = din("wext1", [F_IN, 388], F32R)
    w2_in = din("wext2", [2 * HID, 256], F32R)
    wc1_in = din("wc1ext", [HID, 260], F32R)
    wc2_in = din("wc2ext", [2 * HID, 256], F32R)
    p2w_in = din("p2w", [HID, K6], F32R)
    fc1_in = din("fc1", [HID, HID], F32R)
    fc2_in = din("fc2", [HID, OUT], F32R)
    sel_in = din("sel10", [GKP, G], F32R)
    mz_in = din("mzones", [128, 2], F32R)
    ones_in = din("ones128", [128, 1])
    t2g1T_in = din("t2g1T", [2 * HID, N])
    ig1e_in = din("ig1e", [128, NC])
    g2bb_in = din("g2bb", [128, HID])
    cb2e_in = din("cb2e", [128, NC])
    ig2e_in = din("ig2e", [128, NC])
    t2p1T_in = din("t2p1T", [HID, N])
    ig1p_in = din("ig1p", [128, NC])
    t2p2_in = din("t2p2", [N, K6])
    ig2p_in = din("ig2p", [128, NC])
    t2c1_in = din("t2c1", [GKP, 2 * HID])
    ig1c_in = din("ig1c", [GKP, 1])
    t2c2_in = din("t2c2", [GKP, HID])
    ig2c_in = din("ig2c", [GKP, 1])
    fc1b_in = din("fc1b", [HID, 1])
    fc2b_in = din("fc2b", [OUT, 1])
    eyec_in = din("eyec", [GKP, GKP])
    inveyec_in = din("inveyec", [GKP, GKP])
    zk6_in = din("zk6", [128, NC, K6], F32R)
    z64_in = din("zeros64", [GKP, HID], F32R)

    zl_out = dout("z_local", [G, N, HID], F32R)
    zm_out = dout("z_meso", [G, K, HID], F32R)
    s_out = dout("s", [G, N, K])
    ol_out = dout("out_local", [G, OUT])
    om_out = dout("out_meso", [G, OUT])
    st_out = dout("stats", [16, 1])

    with tile.TileContext(nc) as tc:
        with (
            tc.tile_pool(name="const", bufs=1) as cpool,
            tc.tile_pool(name="madj", bufs=2) as mpool,
            tc.tile_pool(name="pers", bufs=1) as ppool,
            tc.tile_pool(name="row", bufs=1) as rpool,
            tc.tile_pool(name="chunk", bufs=2) as kpool,
            tc.tile_pool(name="attn", bufs=2) as apool,
            tc.tile_pool(name="attnf", bufs=2) as fpool,
            tc.tile_pool(name="small", bufs=1) as spool,
            tc.tile_pool(name="pA", bufs=2, space="PSUM") as psA,
            tc.tile_pool(name="pT", bufs=1, space="PSUM") as psT,
            tc.tile_pool(name="pR", bufs=1, space="PSUM") as psR,
        ):
            DMA = nc.sync.dma_start

            def lc(ap_in, shape, dt=F32, tag=None):
                t = cpool.tile(shape, dt, tag=tag)
                DMA(t[:], ap_in[:])
                return t

            eye = lc(eye_in, [128, 128], F32R, "eye")
            eyef = lc(eyef_in, [128, 128], F32, "eyef")
            epscol = lc(eps_in, [128, 1], F32, "epscol")
            w1 = lc(w1_in, [F_IN, 388], F32R, "w1")
            w2 = cpool.tile([128, 2, 256], F32R, tag="w2")
            for h in range(H):
                DMA(w2[:, h, :], w2_in[h * 128:(h + 1) * 128, :])
            wc1 = lc(wc1_in, [HID, 260], F32R, "wc1")
            wc2 = cpool.tile([128, 2, 256], F32R, tag="wc2")
            for h in range(H):
                DMA(wc2[:, h, :], wc2_in[h * 128:(h + 1) * 128, :])
            p2w = lc(p2w_in, [HID, K6], F32R, "p2w")
            fc1 = lc(fc1_in, [HID, HID], F32R, "fc1")
            fc2 = lc(fc2_in, [HID, OUT], F32R, "fc2")
            sel = lc(sel_in, [GKP, G], F32R, "sel")
            mzones = lc(mz_in, [128, 2], F32R, "mz")
            ones = lc(ones_in, [128, 1], F32, "ones")
            t2g1T = cpool.tile([128, 2, N], F32, tag="t2g1T")
            for h in range(H):
                DMA(t2g1T[:, h, :], t2g1T_in[h * 128:(h + 1) * 128, :])
            ig1e = lc(ig1e_in, [128, NC], F32, "ig1e")
            g2bb = lc(g2bb_in, [128, HID], F32, "g2bb")
            cb2e = lc(cb2e_in, [128, NC], F32, "cb2e")
            ig2e = lc(ig2e_in, [128, NC], F32, "ig2e")
            t2p1T = lc(t2p1T_in, [HID, N], F32, "t2p1T")
            ig1p = lc(ig1p_in, [128, NC], F32, "ig1p")
            t2p2 = cpool.tile([128, NC, K6], F32, tag="t2p2")
            for c in range(NC):
                DMA(t2p2[:, c, :], t2p2_in[c * 128:(c + 1) * 128, :])
            ig2p = lc(ig2p_in, [128, NC], F32, "ig2p")
            t2c1 = lc(t2c1_in, [GKP, 2 * HID], F32, "t2c1")
            ig1c = lc(ig1c_in, [GKP, 1], F32, "ig1c")
            t2c2 = lc(t2c2_in, [GKP, HID], F32, "t2c2")
            ig2c = lc(ig2c_in, [GKP, 1], F32, "ig2c")
            fc1b = lc(fc1b_in, [HID, 1], F32, "fc1b")
            fc2b = lc(fc2b_in, [OUT, 1], F32, "fc2b")
            eyec = lc(eyec_in, [GKP, GKP], F32, "eyec")
            inveyec = lc(inveyec_in, [GKP, GKP], F32, "inveyec")

            stats = cpool.tile([128, 16], F32, tag="stats")
            nc.gpsimd.memset(stats[:], 0.0)
            xc10 = cpool.tile([GKP, HID], F32R, tag="xc10")
            DMA(xc10[:], z64_in[:])
            adjc = cpool.tile([GKP, GKP], F32, tag="adjc")
            nc.gpsimd.memset(adjc[:], 0.0)
            mz_all = cpool.tile([HID, G], F32R, tag="mz_all")

            def to_row(wt, nch, tag):
                """wt [128, nch] chunked column vector -> row tile [1, nch*128]
                in node order n = c*128 + p."""
                tp = psT.tile([nch, 128], F32R, tag="pTt")
                nc.tensor.transpose(tp[:], wt[:], eye[:])
                wtT = spool.tile([nch, 128], F32R, tag=f"{tag}T")
                nc.scalar.copy(wtT[:], tp[:])
                row = spool.tile([1, nch * 128], F32R, tag=f"{tag}R")
                a = wtT[:]
                dst = row[:]
                DMA(AP(dst.tensor, dst.offset, [[nch * 128, 1], [1, nch * 128]]),
                    AP(a.tensor, a.offset, [[128, nch], [1, 128]]))
                return row

            def bcast(row_ap, width, tag):
                """row_ap [1, width] -> [128, width] via gpsimd."""
                out = rpool.tile([128, width], row_ap.dtype, tag=tag)
                nc.gpsimd.partition_broadcast(out[:], row_ap, channels=128)
                return out

            def attention(hhat, es, q, grow, m_tiles, nheads, cdim):
                esr = kpool.tile(list(es.shape), F32R, tag="esr")
                nc.vector.tensor_copy(esr[:], es[:])
                """returns per head (np0, np1, rec_cols [128, NC])."""
                res = []
                for h in range(nheads):
                    dg = bcast(grow[h][0:1, :], N, "dg")
                    np0 = psA.tile([cdim, 512], F32, tag="pAa")
                    np1 = psA.tile([cdim, 512], F32, tag="pAb")
                    rp0 = psR.tile([1, 512], F32, tag="pRa")
                    rp1 = psR.tile([1, 512], F32, tag="pRb")
                    for j in range(NC):
                        zt = apool.tile([128, N], F32, tag="zt")
                        nc.vector.tensor_scalar(
                            zt[:], dg[:], q[:, j, h:h + 1], 1.0, OP.mult, OP.max)
                        ft = fpool.tile([128, N], F32R, tag="ft")
                        eng = nc.gpsimd if j % 2 == 1 else nc.vector
                        eng.tensor_tensor(ft[:], zt[:], m_tiles[j][:], OP.mult)
                        lhs = hhat[:, j, h * cdim:(h + 1) * cdim]
                        st, sp = (j == 0), (j == NC - 1)
                        nc.tensor.matmul(np0[:], lhs, ft[:, 0:512], start=st,
                                         stop=sp)
                        nc.tensor.matmul(np1[:], lhs, ft[:, 512:N], start=st,
                                         stop=sp)
                        nc.tensor.matmul(rp0[:], esr[:, j, h:h + 1],
                                         ft[:, 0:512], start=st, stop=sp)
                        nc.tensor.matmul(rp1[:], esr[:, j, h:h + 1],
                                         ft[:, 512:N], start=st, stop=sp)
                    rrow = spool.tile([1, N], F32, tag="rrow")
                    nc.scalar.copy(rrow[:, 0:512], rp0[:])
                    nc.vector.tensor_copy(rrow[:, 512:N], rp1[:])
                    rcp = psT.tile([128, NC], F32, tag="pTu")
                    for c in range(NC):
                        nc.tensor.transpose(rcp[:, c:c + 1],
                                            rrow[:, c * 128:(c + 1) * 128],
                                            eyef[0:1, 0:1])
                    rcol = spool.tile([128, NC], F32, tag="rcol")
                    nc.vector.tensor_copy(rcol[:], rcp[:])
                    rec = spool.tile([128, NC], F32, tag=f"rec{h}")
                    nc.vector.reciprocal(rec[:], rcol[:])
                    res.append((np0, np1, rec))
                return res

            for g in range(G):
                # ---------- adjacency ----------
                m_tiles = []
                for c in range(NC):
                    mt = mpool.tile([128, N], F32R, tag=f"m{c}")
                    DMA(mt[:], adj_in[g, c * 128:(c + 1) * 128, :])
                    m_tiles.append(mt)
                diag = spool.tile([128, NC], F32R, tag="diag")
                for c in range(NC):
                    src = AP(adj_in.tensor, g * N * N + c * 128 * (N + 1),
                             [[N + 1, 128], [1, 1]])
                    DMA(diag[:, c:c + 1], src)
                rsum = spool.tile([128, NC], F32, tag="rsum")
                for c in range(NC):
                    nc.vector.tensor_reduce(rsum[:, c:c + 1], m_tiles[c][:],
                                            AX.X, OP.add)
                    db = m_tiles[c][:, c * 128:(c + 1) * 128]
                    nc.vector.tensor_tensor(db, db, eye[:], OP.max)
                deg = spool.tile([128, NC], F32, tag="deg")
                nc.vector.scalar_tensor_tensor(deg[:], rsum[:], 1.0, diag[:],
                                               OP.add, OP.subtract)
                dln = spool.tile([128, NC], F32, tag="dln")
                nc.scalar.activation(dln[:], deg[:], F.Ln)
                dcol = spool.tile([128, NC], F32, tag="dcol")
                nc.scalar.activation(dcol[:], dln[:], F.Exp, scale=-0.5)
                nc.vector.tensor_reduce(stats[:, 0 + g:1 + g], rsum[:], AX.X,
                                        OP.add)

                # ---------- x load + transpose ----------
                xT = ppool.tile([F_IN, N], F32R, tag="xT")
                for c in range(NC):
                    xt = kpool.tile([128, F_IN], F32R, tag="xt")
                    DMA(xt[:], x_in[g, c * 128:(c + 1) * 128, :])
                    xp = psT.tile([F_IN, 128], F32R, tag="pTt")
                    nc.tensor.transpose(xp[:], xt[:], eye[:])
                    nc.scalar.copy(xT[:, c * 128:(c + 1) * 128], xp[:])

                # ---------- h_ext = x @ [W1|as|ad|p1W] ----------
                hhat1 = ppool.tile([128, NC, 2 * HID], F32R, tag="hhat1")
                es1 = ppool.tile([128, NC, H], F32, tag="es1")
                q1 = ppool.tile([128, NC, H], F32, tag="q1")
                dxwp = ppool.tile([128, NC, HID], F32R, tag="dxwp")
                for c in range(NC):
                    hp = psA.tile([128, 388], F32, tag="pAa")
                    nc.tensor.matmul(hp[:], xT[:, c * 128:(c + 1) * 128], w1[:],
                                     start=True, stop=True)
                    nc.scalar.activation(es1[:, c, :], hp[:, 256:258], F.Exp)
                    nc.scalar.activation(q1[:, c, :], hp[:, 256:258], F.Exp,
                                         scale=-0.8)
                    for h in range(H):
                        nc.vector.tensor_scalar(
                            hhat1[:, c, h * HID:(h + 1) * HID],
                            hp[:, h * HID:(h + 1) * HID], es1[:, c, h:h + 1],
                            None, OP.mult)
                    nc.vector.tensor_scalar(dxwp[:, c, :], hp[:, 260:388],
                                            dcol[:, c:c + 1], None, OP.mult)

                g1rows = []
                for h in range(H):
                    adp0 = psR.tile([1, 512], F32, tag="pRa")
                    adp1 = psR.tile([1, 512], F32, tag="pRb")
                    nc.tensor.matmul(adp0[:], w1[:, 258 + h:259 + h],
                                     xT[:, 0:512], start=True, stop=True)
                    nc.tensor.matmul(adp1[:], w1[:, 258 + h:259 + h],
                                     xT[:, 512:N], start=True, stop=True)
                    g1rowh = spool.tile([1, N], F32, tag=f"g1row{h}")
                    nc.scalar.activation(g1rowh[:, 0:512], adp0[:], F.Exp,
                                         scale=-0.8)
                    nc.scalar.activation(g1rowh[:, 512:N], adp1[:], F.Exp,
                                         scale=-0.8)
                    g1rows.append(g1rowh)

                # ---------- GAT1 ----------
                att1 = attention(hhat1, es1, q1, g1rows, m_tiles, H, HID)
                z1T = []
                for h in range(H):
                    np0, np1, rec = att1[h]
                    wt = spool.tile([128, NC], F32R, tag="wt")
                    nc.vector.tensor_tensor(wt[:], rec[:], ig1e[:], OP.mult)
                    wrow = to_row(wt, NC, "w1h")
                    wb = bcast(wrow[:], N, "wbr")
                    zT = ppool.tile([128, N], F32R, tag=f"z1T{h}")
                    for i2 in range(2):
                        sl = slice(i2 * 512, (i2 + 1) * 512)
                        npp = np0 if i2 == 0 else np1
                        u = kpool.tile([128, 512], F32, tag="u1")
                        nc.vector.scalar_tensor_tensor(
                            u[:], npp[:], 1.0, wb[:, sl], OP.mult, OP.mult)
                        v = kpool.tile([128, 512], F32, tag="v1")
                        eng = nc.gpsimd if i2 == 1 else nc.vector
                        eng.tensor_tensor(v[:], u[:],
                                          t2g1T[:, h, sl],
                                          OP.add)
                        nc.scalar.activation(zT[:, sl], v[:], F.Relu)
                    z1T.append(zT)

                # ---------- GAT2 ----------
                hhat2 = ppool.tile([128, NC, HID], F32R, tag="hhat2")
                es2 = ppool.tile([128, NC, 1], F32, tag="es2")
                q2 = ppool.tile([128, NC, 1], F32, tag="q2")
                for c in range(NC):
                    h2p = psA.tile([128, 256], F32, tag="pAb")
                    for h in range(H):
                        nc.tensor.matmul(h2p[:],
                                         z1T[h][:, c * 128:(c + 1) * 128],
                                         w2[:, h, :],
                                         start=(h == 0), stop=(h == 1))
                    nc.scalar.activation(es2[:, c, :], h2p[:, HID:HID + 1],
                                         F.Exp)
                    nc.scalar.activation(q2[:, c, :], h2p[:, HID:HID + 1],
                                         F.Exp, scale=-0.8)
                    nc.vector.tensor_scalar(hhat2[:, c, :], h2p[:, 0:HID],
                                            es2[:, c, 0:1], None, OP.mult)
                ad20 = psR.tile([1, 512], F32, tag="pRa")
                ad21 = psR.tile([1, 512], F32, tag="pRb")
                for h in range(H):
                    st, sp = (h == 0), (h == 1)
                    nc.tensor.matmul(ad20[:],
                                     w2[:, h, HID + 1:HID + 2],
                                     z1T[h][:, 0:512], start=st, stop=sp)
                    nc.tensor.matmul(ad21[:],
                                     w2[:, h, HID + 1:HID + 2],
                                     z1T[h][:, 512:N], start=st, stop=sp)
                g2row = spool.tile([1, N], F32, tag="g2row")
                nc.scalar.activation(g2row[:, 0:512], ad20[:], F.Exp, scale=-0.8)
                nc.scalar.activation(g2row[:, 512:N], ad21[:], F.Exp, scale=-0.8)

                att2 = attention(hhat2, es2, q2, [g2row], m_tiles, 1, HID)
                np0, np1, rec2 = att2[0]
                zl = ppool.tile([128, NC, HID], F32R, tag="zl")
                for c in range(NC):
                    npp = np0 if c < 4 else np1
                    off = (c % 4) * 128
                    tsb = kpool.tile([128, 128], F32R, tag="tsb")
                    nc.scalar.copy(tsb[:], npp[:, off:off + 128])
                    tp = psT.tile([128, 128], F32R, tag="pTt")
                    nc.tensor.transpose(tp[:], tsb[:], eye[:])
                    zraw = kpool.tile([128, HID], F32, tag="zraw")
                    nc.vector.scalar_tensor_tensor(
                        zraw[:], tp[:], rec2[:, c:c + 1],
                        g2bb[:], OP.mult, OP.add)
                    v2 = kpool.tile([128, HID], F32, tag="v2")
                    nc.vector.tensor_scalar(v2[:], zraw[:], 0.0, None, OP.max)
                    nc.scalar.activation(zl[:, c, :], v2[:], F.Relu,
                                         scale=ig2e[:, c:c + 1],
                                         bias=cb2e[:, c:c + 1])
                    DMA(zl_out[g, c * 128:(c + 1) * 128, :], zl[:, c, :])

                # ---------- pool branch GCN1 (flipped) ----------
                gp0 = psA.tile([HID, 512], F32, tag="pAa")
                gp1 = psA.tile([HID, 512], F32, tag="pAb")
                for j in range(NC):
                    st, sp = (j == 0), (j == NC - 1)
                    nc.tensor.matmul(gp0[:], dxwp[:, j, :], m_tiles[j][:, 0:512],
                                     start=st, stop=sp)
                    nc.tensor.matmul(gp1[:], dxwp[:, j, :], m_tiles[j][:, 512:N],
                                     start=st, stop=sp)
                wpt = spool.tile([128, NC], F32R, tag="wpt")
                nc.vector.tensor_tensor(wpt[:], dcol[:], ig1p[:], OP.mult)
                wprow = to_row(wpt, NC, "wp")
                wpb = bcast(wprow[:], N, "wbr")
                s1T = ppool.tile([HID, N], F32R, tag="s1T")
                for i2 in range(2):
                    sl = slice(i2 * 512, (i2 + 1) * 512)
                    npp = gp0 if i2 == 0 else gp1
                    u = kpool.tile([128, 512], F32, tag="u1")
                    nc.vector.scalar_tensor_tensor(
                        u[:], npp[:], 1.0, wpb[:, sl], OP.mult, OP.mult)
                    v = kpool.tile([128, 512], F32, tag="v1")
                    nc.gpsimd.tensor_tensor(v[:], u[:], t2p1T[:, sl], OP.add)
                    nc.scalar.activation(s1T[:, sl], v[:], F.Relu)

                # ---------- GCN2 ----------
                dsw = ppool.tile([128, NC, K6], F32R, tag="dsw")
                for c in range(NC):
                    swp = psT.tile([128, K6], F32, tag="pTt")
                    nc.tensor.matmul(swp[:], s1T[:, c * 128:(c + 1) * 128],
                                     p2w[:], start=True, stop=True)
                    nc.vector.tensor_scalar(dsw[:, c, :], swp[:],
                                            dcol[:, c:c + 1], None, OP.mult)
                sp0 = psA.tile([K6, 512], F32, tag="pAa")
                sp1 = psA.tile([K6, 512], F32, tag="pAb")
                for j in range(NC):
                    st, sp_ = (j == 0), (j == NC - 1)
                    nc.tensor.matmul(sp0[:], dsw[:, j, :], m_tiles[j][:, 0:512],
                                     start=st, stop=sp_)
                    nc.tensor.matmul(sp1[:], dsw[:, j, :], m_tiles[j][:, 512:N],
                                     start=st, stop=sp_)
                s2T = spool.tile([K6, N], F32R, tag="s2T")
                nc.scalar.copy(s2T[:, 0:512], sp0[:])
                nc.vector.tensor_copy(s2T[:, 512:N], sp1[:])
                wp2 = spool.tile([128, NC], F32, tag="wp2")
                nc.vector.tensor_tensor(wp2[:], dcol[:], ig2p[:], OP.mult)
                slog = ppool.tile([128, NC, K6], F32, tag="slog")
                for c in range(NC):
                    tpk = psT.tile([128, K6], F32R, tag="pTt")
                    nc.tensor.transpose(tpk[:], s2T[:, c * 128:(c + 1) * 128],
                                        eye[0:K6, 0:K6])
                    nc.vector.scalar_tensor_tensor(
                        slog[:, c, :], tpk[:], wp2[:, c:c + 1],
                        t2p2[:, c, :], OP.mult, OP.add)

                # ---------- double softmax over k ----------
                s_f = ppool.tile([128, NC, K6], F32R, tag="s_f")
                DMA(s_f[:], zk6_in[:])
                cur = slog[:, :, 0:K]
                for rep in range(2):
                    mx = kpool.tile([128, NC], F32, tag="mx")
                    nc.vector.tensor_reduce(mx[:], cur, AX.X, OP.max,
                                            negate=True)
                    ex = kpool.tile([128, NC, K], F32, tag="ex")
                    nc.vector.scalar_tensor_tensor(
                        ex[:], cur, 1.0, mx[:].broadcast_to([128, NC, K]),
                        OP.mult, OP.add)
                    ex2 = kpool.tile([128, NC, K], F32, tag="ex2")
                    nc.scalar.activation(ex2[:], ex[:], F.Exp)
                    sm = kpool.tile([128, NC], F32, tag="sm")
                    nc.vector.tensor_reduce(sm[:], ex2[:], AX.X, OP.add)
                    rc = kpool.tile([128, NC], F32, tag="rc")
                    nc.vector.reciprocal(rc[:], sm[:])
                    dst = kpool.tile([128, NC, K], F32, tag="smid")
                    nc.vector.scalar_tensor_tensor(
                        dst[:], ex2[:], 1.0, rc[:].broadcast_to([128, NC, K]),
                        OP.mult, OP.mult)
                    if rep == 1:
                        nc.vector.tensor_copy(s_f[:, :, 0:K], dst[:])
                    if rep == 0:
                        for c in range(NC):
                            DMA(s_out[g, c * 128:(c + 1) * 128, :], dst[:, c, :])
                    cur = dst[:]

                # ---------- ent loss ----------
                lg = kpool.tile([128, NC, K], F32, tag="lg")
                nc.scalar.activation(lg[:], s_f[:, :, 0:K], F.Ln,
                                     bias=epscol[:, 0:1])
                lg2 = kpool.tile([128, NC, K], F32, tag="lg2")
                entc = spool.tile([128, 1], F32, tag="entc")
                nc.vector.scalar_tensor_tensor(lg2[:], lg[:], 1.0,
                                               s_f[:, :, 0:K],
                                               OP.mult, OP.mult,
                                               accum_out=entc[:])
                nc.vector.tensor_scalar(stats[:, 2 + g:3 + g], entc[:], -1.0,
                                        None, OP.mult)

                # ---------- diffpool ----------
                ap0 = psA.tile([K6, 512], F32, tag="pAa")
                ap1 = psA.tile([K6, 512], F32, tag="pAb")
                for j in range(NC):
                    st, sp_ = (j == 0), (j == NC - 1)
                    nc.tensor.matmul(ap0[:], s_f[:, j, :],
                                     m_tiles[j][:, 0:512], start=st, stop=sp_)
                    nc.tensor.matmul(ap1[:], s_f[:, j, :],
                                     m_tiles[j][:, 512:N], start=st, stop=sp_)
                sTA = spool.tile([K6, N], F32R, tag="sTA")
                nc.scalar.copy(sTA[:, 0:512], ap0[:])
                nc.vector.tensor_copy(sTA[:, 512:N], ap1[:])
                vAs = ppool.tile([128, NC, K6], F32R, tag="vAs")
                for c in range(NC):
                    tpk = psT.tile([128, K6], F32R, tag="pTt")
                    nc.tensor.transpose(tpk[:], sTA[:, c * 128:(c + 1) * 128],
                                        eye[0:K6, 0:K6])
                    nc.vector.tensor_copy(vAs[:, c, :], tpk[:])
                wdg = spool.tile([128, NC], F32, tag="wdg")
                nc.vector.tensor_scalar(wdg[:], diag[:], -1.0, 1.0, OP.mult,
                                        OP.add)
                sw_ = ppool.tile([128, NC, K6], F32R, tag="sw_")
                for c in range(NC):
                    nc.vector.tensor_scalar(sw_[:, c, :], s_f[:, c, :],
                                            wdg[:, c:c + 1], None, OP.mult)
                oasb = spool.tile([K6, 2 * K6 + HID], F32, tag="oasb")
                corr = spool.tile([K6, K6], F32, tag="corr")
                for grp in range(4):
                    gp = psT.tile([K6, 2 * K6 + HID], F32, tag="pTt")
                    rng_ = [(0, K6), (K6, 2 * K6), (2 * K6, 2 * K6 + HID),
                            (0, K6)][grp]
                    for c in range(NC):
                        st, sp_ = (c == 0), (c == NC - 1)
                        lhs = sw_[:, c, :] if grp == 3 else s_f[:, c, :]
                        rhs = [vAs[:, c, :], s_f[:, c, :], zl[:, c, :],
                               s_f[:, c, :]][grp]
                        nc.tensor.matmul(gp[:, rng_[0]:rng_[1]], lhs, rhs,
                                         start=st, stop=sp_)
                    if grp == 3:
                        nc.scalar.copy(corr[:], gp[:, 0:K6])
                    else:
                        nc.scalar.copy(oasb[:, rng_[0]:rng_[1]],
                                       gp[:, rng_[0]:rng_[1]])
                oadj = spool.tile([K, K], F32, tag="oadj")
                nc.vector.tensor_tensor(oadj[:], oasb[0:K, 0:K],
                                        corr[0:K, 0:K], OP.subtract)
                tr2 = spool.tile([K, K], F32, tag="tr2")
                nc.vector.tensor_tensor(tr2[:], oadj[:], eyec[0:K, 0:K],
                                        OP.mult)
                nc.vector.tensor_reduce(stats[0:K, 4 + g:5 + g], tr2[:], AX.X,
                                        OP.add)
                stsq = spool.tile([K, K], F32, tag="stsq")
                nc.vector.tensor_tensor(stsq[:], oasb[0:K, K6:K6 + K],
                                        oasb[0:K, K6:K6 + K], OP.mult)
                nc.vector.tensor_reduce(stats[0:K, 6 + g:7 + g], stsq[:], AX.X,
                                        OP.add)
                nc.vector.tensor_copy(xc10[g * 32:g * 32 + K, :],
                                      oasb[0:K, 2 * K6:])
                nc.vector.tensor_copy(adjc[g * 32:g * 32 + K,
                                           g * 32:g * 32 + K], oadj[:])

                # ---------- out_local mean ----------
                mzp = psT.tile([HID, 2], F32, tag="pTu")
                for c in range(NC):
                    nc.tensor.matmul(mzp[:], zl[:, c, :], mzones[:],
                                     start=(c == 0), stop=(c == NC - 1))
                nc.scalar.copy(mz_all[:, g:g + 1], mzp[:, 0:1])

            # ================= coarse branch (both graphs, 10 rows) =========
            adjcsl = spool.tile([GKP, GKP], F32, tag="adjcsl")
            nc.vector.tensor_tensor(adjcsl[:], adjc[:], inveyec[:], OP.mult)
            nc.vector.tensor_tensor(adjcsl[:], adjcsl[:], eyec[:], OP.add)
            maskc = spool.tile([GKP, GKP], F32, tag="maskc")
            nc.vector.tensor_scalar(maskc[:], adjcsl[:], 0.0, None,
                                    OP.not_equal)

            def coarse_gat(xin_t, wsl, nheads, cdim, t2, igc, zname):
                nin = xin_t.shape[1]
                nf = nheads * cdim
                wcols = wsl[0].shape[-1]
                nchunks = (nin + 127) // 128
                xtp = []
                for h2 in range(nchunks):
                    w = min(128, nin - h2 * 128)
                    tp = psT.tile([128, GKP], F32R, tag="pTt")
                    nc.tensor.transpose(tp[0:w, :],
                                        xin_t[:, h2 * 128:h2 * 128 + w],
                                        eye[0:GKP, 0:GKP])
                    xt = spool.tile([128, GKP], F32R, tag=f"{zname}xT{h2}")
                    nc.scalar.copy(xt[0:w, :], tp[0:w, :])
                    xtp.append((xt, w))
                hcp = psT.tile([GKP, wcols], F32, tag="pTu")
                for i, (xt, w) in enumerate(xtp):
                    nc.tensor.matmul(hcp[:], xt[0:w, :], wsl[i],
                                     start=(i == 0), stop=(i == nchunks - 1))
                hc = spool.tile([GKP, wcols], F32R, tag=f"{zname}hc")
                nc.scalar.copy(hc[:], hcp[:])
                esc = spool.tile([GKP, nheads], F32, tag=f"{zname}es")
                nc.scalar.activation(esc[:], hc[:, nf:nf + nheads], F.Exp)
                qc = spool.tile([GKP, nheads], F32, tag=f"{zname}q")
                nc.scalar.activation(qc[:], hc[:, nf:nf + nheads], F.Exp,
                                     scale=-0.8)
                gcrs = []
                for h in range(nheads):
                    adps = psT.tile([1, GKP], F32, tag="pTt")
                    for i, (xt, w) in enumerate(xtp):
                        nc.tensor.matmul(
                            adps[:],
                            wsl[i][:, nf + nheads + h:nf + nheads + h + 1],
                            xt[0:w, :], start=(i == 0), stop=(i == nchunks - 1))
                    gcrh = spool.tile([1, GKP], F32, tag=f"{zname}gr{h}")
                    nc.scalar.activation(gcrh[:], adps[:], F.Exp, scale=-0.8)
                    gcrs.append(gcrh)
                zc = spool.tile([GKP, nf], F32R, tag=f"{zname}z")
                for h in range(nheads):
                    hhc = spool.tile([GKP, cdim], F32R, tag=f"{zname}hh")
                    nc.vector.tensor_scalar(hhc[:],
                                            hc[:, h * cdim:(h + 1) * cdim],
                                            esc[:, h:h + 1], None, OP.mult)
                    dgc = spool.tile([GKP, GKP], F32, tag=f"{zname}dg")
                    nc.gpsimd.partition_broadcast(dgc[:], gcrs[h][0:1, :],
                                                  channels=GKP)
                    zcc = spool.tile([GKP, GKP], F32, tag=f"{zname}zc")
                    nc.vector.tensor_scalar(zcc[:], dgc[:], qc[:, h:h + 1], 1.0,
                                            OP.mult, OP.max)
                    fcc = spool.tile([GKP, GKP], F32R, tag=f"{zname}fc")
                    nc.vector.tensor_tensor(fcc[:], zcc[:], maskc[:], OP.mult)
                    nump = psT.tile([cdim, GKP], F32, tag="pTt")
                    nc.tensor.matmul(nump[:], hhc[:], fcc[:], start=True,
                                     stop=True)
                    escr = spool.tile([GKP, 1], F32R, tag=f"{zname}esr")
                    nc.vector.tensor_copy(escr[:], esc[:, h:h + 1])
                    rpc = psT.tile([1, GKP], F32, tag="pTu")
                    nc.tensor.matmul(rpc[:], escr[:], fcc[:], start=True,
                                     stop=True)
                    nsb = spool.tile([cdim, GKP], F32R, tag=f"{zname}nsb")
                    nc.scalar.copy(nsb[:], nump[:])
                    rsb = spool.tile([1, GKP], F32, tag=f"{zname}rsb")
                    nc.scalar.copy(rsb[:], rpc[:])
                    ntp = psT.tile([GKP, cdim], F32R, tag="pTt")
                    nc.tensor.transpose(ntp[:], nsb[:], eye[:])
                    rtp = psT.tile([GKP, 1], F32, tag="pTu")
                    nc.tensor.transpose(rtp[:], rsb[:], eyef[0:1, 0:1])
                    rcc = spool.tile([GKP, 1], F32, tag=f"{zname}rcc")
                    nc.vector.tensor_copy(rcc[:], rtp[:])
                    recc = spool.tile([GKP, 1], F32, tag=f"{zname}recc")
                    nc.vector.reciprocal(recc[:], rcc[:])
                    wcc = spool.tile([GKP, 1], F32, tag=f"{zname}wcc")
                    nc.vector.tensor_tensor(wcc[:], recc[:], igc[:], OP.mult)
                    zpre = spool.tile([GKP, cdim], F32, tag=f"{zname}zpre")
                    nc.vector.scalar_tensor_tensor(
                        zpre[:], ntp[:], wcc[:, 0:1],
                        t2[:, h * cdim:(h + 1) * cdim], OP.mult, OP.add)
                    nc.scalar.activation(zc[:, h * cdim:(h + 1) * cdim],
                                         zpre[:], F.Relu)
                return zc

            z1c = coarse_gat(xc10, [wc1[:]], H, HID, t2c1, ig1c, "c1")
            zmeso = coarse_gat(z1c, [wc2[:, 0, :], wc2[:, 1, :]], 1, HID, t2c2, ig2c, "c2")
            for g in range(G):
                DMA(zm_out[g], zmeso[g * 32:g * 32 + K, :])

            # ---------- readout ----------
            mmp = psT.tile([HID, G], F32, tag="pTt")
            nc.tensor.matmul(mmp[:], zmeso[:], sel[:], start=True, stop=True)
            mmT = spool.tile([HID, G], F32R, tag="mmT")
            nc.scalar.copy(mmT[:], mmp[:])
            omp = psT.tile([HID, G], F32, tag="pTu")
            nc.tensor.matmul(omp[:], fc1[:], mmT[:], start=True, stop=True)
            omT = spool.tile([HID, G], F32R, tag="omT")
            nc.scalar.activation(omT[:], omp[:], F.Relu, bias=fc1b[:, 0:1])
            om2p = psT.tile([OUT, G], F32, tag="pTt")
            nc.tensor.matmul(om2p[:], fc2[:], omT[:], start=True, stop=True)
            omf = spool.tile([OUT, G], F32, tag="omf")
            nc.vector.tensor_scalar(omf[:], om2p[:], fc2b[:, 0:1], None, OP.add)
            olp = psT.tile([OUT, G], F32, tag="pTu")
            nc.tensor.matmul(olp[:], fc2[:], mz_all[:], start=True, stop=True)
            olf = spool.tile([OUT, G], F32, tag="olf")
            nc.vector.tensor_scalar(olf[:], olp[:], fc2b[:, 0:1], None, OP.add)
            aom = om_out[:]
            DMA(AP(aom.tensor, aom.offset, [[1, OUT], [OUT, G]]), omf[:])
            aol = ol_out[:]
            DMA(AP(aol.tensor, aol.offset, [[1, OUT], [OUT, G]]), olf[:])

            # ---------- stats ----------
            stp = psT.tile([16, 1], F32, tag="pTu")
            nc.tensor.matmul(stp[:], stats[:], ones[:], start=True, stop=True)
            stsb = spool.tile([16, 1], F32, tag="stsb")
            nc.scalar.copy(stsb[:], stp[:])
            DMA(st_out[:], stsb[:])

    nc.compile()
    _cache['nc'] = nc
    return nc


def _fold_params(p):
    d = {k: np.asarray(v, np.float64) for k, v in p.items()}

    def bnfold(pre):
        ig = d[pre + '_g'] / np.sqrt(d[pre + '_v'] + BN_EPS)
        c = d[pre + '_b'] - d[pre + '_m'] * ig
        return ig, c

    out = {}
    g1W = d['g1_W']
    was1 = np.stack([g1W.reshape(F_IN, H, HID)[:, h, :] @ d['g1_as'][h]
                     for h in range(H)], 1)
    wad1 = np.stack([g1W.reshape(F_IN, H, HID)[:, h, :] @ d['g1_ad'][h]
                     for h in range(H)], 1)
    out['wext1'] = np.concatenate([g1W, was1, wad1, d['p1_W']], 1)
    was2 = d['g2_W'] @ d['g2_as'][0]
    wad2 = d['g2_W'] @ d['g2_ad'][0]
    out['wext2'] = np.concatenate(
        [d['g2_W'], was2[:, None], wad2[:, None], np.zeros((2 * HID, 126))], 1)
    c1W = d['c1_W']
    wasc = np.stack([c1W.reshape(HID, H, HID)[:, h, :] @ d['c1_as'][h]
                     for h in range(H)], 1)
    wadc = np.stack([c1W.reshape(HID, H, HID)[:, h, :] @ d['c1_ad'][h]
                     for h in range(H)], 1)
    out['wc1ext'] = np.concatenate([c1W, wasc, wadc], 1)
    wasc2 = d['c2_W'] @ d['c2_as'][0]
    wadc2 = d['c2_W'] @ d['c2_ad'][0]
    out['wc2ext'] = np.concatenate(
        [d['c2_W'], wasc2[:, None], wadc2[:, None], np.zeros((2 * HID, 126))],
        1)
    out['p2w'] = np.concatenate([d['p2_W'], np.zeros((HID, 1))], 1)
    out['fc1'] = d['fc1_W']
    out['fc2'] = d['fc2_W']
    out['fc1b'] = d['fc1_b'][:, None]
    out['fc2b'] = d['fc2_b'][:, None]

    ig1e, c1e = bnfold('bn1e')
    ig2e, c2e = bnfold('bn2e')
    ig1p, c1p = bnfold('bn1p')
    ig2p, c2p = bnfold('bn2p')
    ig1c, c1c = bnfold('bn1c')
    ig2c, c2c = bnfold('bn2c')

    def cols(v):
        return np.ascontiguousarray(v.reshape(NC, 128).T)

    out['ig1e'] = cols(ig1e)
    out['ig2e'] = cols(ig2e)
    out['ig1p'] = cols(ig1p)
    out['ig2p'] = cols(ig2p)
    out['t2g1T'] = np.outer(d['g1_b'], ig1e) + c1e[None, :]
    out['g2bb'] = np.tile(d['g2_b'][None, :], (128, 1))
    out['cb2e'] = cols(c2e)
    out['t2p1T'] = np.outer(d['p1_b'], ig1p) + c1p[None, :]
    out['t2p2'] = np.concatenate([np.outer(ig2p, d['p2_b']) + c2p[:, None], np.zeros((N, 1))], 1)
    t2c1 = np.zeros((64, 2 * HID))
    t2c2 = np.zeros((64, HID))
    ig1cp = np.zeros((64, 1))
    ig2cp = np.zeros((64, 1))
    for g in range(G):
        t2c1[g * 32:g * 32 + K] = np.outer(ig1c, d['c1_b']) + c1c[:, None]
        t2c2[g * 32:g * 32 + K] = np.outer(ig2c, d['c2_b']) + c2c[:, None]
        ig1cp[g * 32:g * 32 + K, 0] = ig1c
        ig2cp[g * 32:g * 32 + K, 0] = ig2c
    out['t2c1'] = t2c1
    out['t2c2'] = t2c2
    out['ig1c'] = ig1cp
    out['ig2c'] = ig2cp

    out['eye128'] = np.eye(128)
    out['eye128f'] = np.eye(128)
    out['epscol'] = np.full((128, 1), 1e-15)
    out['zk6'] = np.zeros((128, NC, 6))
    out['zeros64'] = np.zeros((64, HID))
    out['eyec'] = np.eye(64)
    out['inveyec'] = 1.0 - np.eye(64)
    sel = np.zeros((64, G))
    for g in range(G):
        sel[g * 32:g * 32 + K, g] = 1.0 / K
    out['sel10'] = sel
    out['mzones'] = np.concatenate([np.full((128, 1), 1.0 / N), np.zeros((128, 1))], 1)
    out['ones128'] = np.ones((128, 1))
    return {k: np.ascontiguousarray(v, dtype=np.float32)
            for k, v in out.items()}


def kernel(x_dense, adj_dense, params):
    x = np.ascontiguousarray(np.asarray(x_dense), np.float32)
    adj = np.ascontiguousarray(np.asarray(adj_dense), np.float32)
    pf = _fold_params({k: np.asarray(v) for k, v in params.items()})
    nc = build_program()
    in_maps = []
    for core in range(NCORES):
        m = dict(pf)
        m['x'] = x[core * G:(core + 1) * G]
        m['adj'] = adj[core * G:(core + 1) * G]
        in_maps.append(m)
    res = run_bass_kernel_spmd(nc, in_maps, core_ids=list(range(NCORES)))
    z_local = np.concatenate([r['z_local'] for r in res.results], 0)
    z_meso = np.concatenate([r['z_meso'] for r in res.results], 0)
    s = np.concatenate([r['s'] for r in res.results], 0)
    out_local = np.concatenate([r['out_local'] for r in res.results], 0)
    out_meso = np.concatenate([r['out_meso'] for r in res.results], 0)
    link_sq = 0.0
    ent_sum = 0.0
    for r in res.results:
        st = r['stats'][:, 0].astype(np.float64)
        for g in range(G):
            link_sq += st[0 + g] - 2.0 * st[4 + g] + st[6 + g]
            ent_sum += st[2 + g]
    link_loss = np.float32(np.sqrt(link_sq) / (B * N * N))
    ent_loss = np.float32(ent_sum / (B * N))
    return (z_local, z_meso, s, out_local, out_meso, link_loss, ent_loss)


# revision 29
# speedup vs baseline: 4713.8337x; 1.0044x over previous
"""DiffPool forward on 8 Trainium2 NeuronCores, data-parallel over batch.

B=16 graphs -> 2 per core; identical Bass program per core; host folds params,
shards inputs, combines device-computed loss partial sums.

Masked GAT softmax uses
  exp(leaky_relu(s_j + d_i)) = max(exp(s_j)exp(d_i), exp(.2 s_j)exp(.2 d_i));
factoring exp(d_i) (cancels against the softmax row sum) leaves
  F[j,i] = exp(s_j) * max(1, q_j g_i) * mask[j,i],
  q_j = exp(-.8 s_j), g_i = exp(-.8 d_i)
so no transcendental touches an [N,N] tile.

link_loss: sum((adj - s s^T)^2) = sum(adj) - 2 tr(s^T adj s) + |s^T s|_F^2.
"""

import sys

sys.path.insert(0, '/opt/trn_rl_repo')

import numpy as np

import concourse.bacc as bacc
import concourse.mybir as mybir
from concourse import tile
from concourse.bass import AP
from concourse.bass_utils import run_bass_kernel_spmd

F = mybir.ActivationFunctionType
OP = mybir.AluOpType
AX = mybir.AxisListType
F32 = mybir.dt.float32
F32R = mybir.dt.float32r

B, N, F_IN, HID, OUT, K, H = 16, 1024, 64, 128, 8, 5, 2
BN_EPS = 1e-5
NCORES = 8
G = B // NCORES
NC = N // 128
GK = G * K
GKP = 64
K6 = 6

_cache = {}


def build_program():
    if 'nc' in _cache:
        return _cache['nc']
    nc = bacc.Bacc(None, target_bir_lowering=False, debug=False)

    def din(name, shape, dt=F32):
        return nc.dram_tensor(name, shape, dt, kind="ExternalInput").ap()

    def dout(name, shape, dt=F32):
        return nc.dram_tensor(name, shape, dt, kind="ExternalOutput").ap()

    x_in = din("x", [G, N, F_IN], F32R)
    adj_in = din("adj", [G, N, N], F32R)
    eye_in = din("eye128", [128, 128], F32R)
    eyef_in = din("eye128f", [128, 128])
    eps_in = din("epscol", [128, 1])
    w1_in = din("wext1", [F_IN, 388], F32R)
    w2_in = din("wext2", [2 * HID, 256], F32R)
    wc1_in = din("wc1ext", [HID, 260], F32R)
    wc2_in = din("wc2ext", [2 * HID, 256], F32R)
    p2w_in = din("p2w", [HID, K6], F32R)
    fc1_in = din("fc1", [HID, HID], F32R)
    fc2_in = din("fc2", [HID, OUT], F32R)
    sel_in = din("sel10", [GKP, G], F32R)
    mz_in = din("mzones", [128, 2], F32R)
    ones_in = din("ones128", [128, 1])
    t2g1T_in = din("t2g1T", [2 * HID, N])
    ig1e_in = din("ig1e", [128, NC])
    g2bb_in = din("g2bb", [128, HID])
    cb2e_in = din("cb2e", [128, NC])
    ig2e_in = din("ig2e", [128, NC])
    t2p1T_in = din("t2p1T", [HID, N])
    ig1p_in = din("ig1p", [128, NC])
    t2p2_in = din("t2p2", [N, K6])
    ig2p_in = din("ig2p", [128, NC])
    t2c1_in = din("t2c1", [GKP, 2 * HID])
    ig1c_in = din("ig1c", [GKP, 1])
    t2c2_in = din("t2c2", [GKP, HID])
    ig2c_in = din("ig2c", [GKP, 1])
    fc1b_in = din("fc1b", [HID, 1])
    fc2b_in = din("fc2b", [OUT, 1])
    eyec_in = din("eyec", [GKP, GKP])
    inveyec_in = din("inveyec", [GKP, GKP])
    zk6_in = din("zk6", [128, NC, K6], F32R)
    z64_in = din("zeros64", [GKP, HID], F32R)

    zl_out = dout("z_local", [G, N, HID], F32R)
    zm_out = dout("z_meso", [G, K, HID], F32R)
    s_out = dout("s", [G, N, K])
    ol_out = dout("out_local", [G, OUT])
    om_out = dout("out_meso", [G, OUT])
    st_out = dout("stats", [16, 1])

    with tile.TileContext(nc) as tc:
        with (
            tc.tile_pool(name="const", bufs=1) as cpool,
            tc.tile_pool(name="madj", bufs=2) as mpool,
            tc.tile_pool(name="pers", bufs=1) as ppool,
            tc.tile_pool(name="row", bufs=1) as rpool,
            tc.tile_pool(name="chunk", bufs=2) as kpool,
            tc.tile_pool(name="attn", bufs=2) as apool,
            tc.tile_pool(name="attnf", bufs=2) as fpool,
            tc.tile_pool(name="small", bufs=1) as spool,
            tc.tile_pool(name="pA", bufs=2, space="PSUM") as psA,
            tc.tile_pool(name="pT", bufs=1, space="PSUM") as psT,
            tc.tile_pool(name="pR", bufs=1, space="PSUM") as psR,
        ):
            DMA = nc.sync.dma_start

            def lc(ap_in, shape, dt=F32, tag=None):
                t = cpool.tile(shape, dt, tag=tag)
                DMA(t[:], ap_in[:])
                return t

            eye = lc(eye_in, [128, 128], F32R, "eye")
            eyef = lc(eyef_in, [128, 128], F32, "eyef")
            epscol = lc(eps_in, [128, 1], F32, "epscol")
            w1 = lc(w1_in, [F_IN, 388], F32R, "w1")
            w2 = cpool.tile([128, 2, 256], F32R, tag="w2")
            for h in range(H):
                DMA(w2[:, h, :], w2_in[h * 128:(h + 1) * 128, :])
            wc1 = lc(wc1_in, [HID, 260], F32R, "wc1")
            wc2 = cpool.tile([128, 2, 256], F32R, tag="wc2")
            for h in range(H):
                DMA(wc2[:, h, :], wc2_in[h * 128:(h + 1) * 128, :])
            p2w = lc(p2w_in, [HID, K6], F32R, "p2w")
            fc1 = lc(fc1_in, [HID, HID], F32R, "fc1")
            fc2 = lc(fc2_in, [HID, OUT], F32R, "fc2")
            sel = lc(sel_in, [GKP, G], F32R, "sel")
            mzones = lc(mz_in, [128, 2], F32R, "mz")
            ones = lc(ones_in, [128, 1], F32, "ones")
            t2g1T = cpool.tile([128, 2, N], F32, tag="t2g1T")
            for h in range(H):
                DMA(t2g1T[:, h, :], t2g1T_in[h * 128:(h + 1) * 128, :])
            ig1e = lc(ig1e_in, [128, NC], F32, "ig1e")
            g2bb = lc(g2bb_in, [128, HID], F32, "g2bb")
            cb2e = lc(cb2e_in, [128, NC], F32, "cb2e")
            ig2e = lc(ig2e_in, [128, NC], F32, "ig2e")
            t2p1T = lc(t2p1T_in, [HID, N], F32, "t2p1T")
            ig1p = lc(ig1p_in, [128, NC], F32, "ig1p")
            t2p2 = cpool.tile([128, NC, K6], F32, tag="t2p2")
            for c in range(NC):
                DMA(t2p2[:, c, :], t2p2_in[c * 128:(c + 1) * 128, :])
            ig2p = lc(ig2p_in, [128, NC], F32, "ig2p")
            t2c1 = lc(t2c1_in, [GKP, 2 * HID], F32, "t2c1")
            ig1c = lc(ig1c_in, [GKP, 1], F32, "ig1c")
            t2c2 = lc(t2c2_in, [GKP, HID], F32, "t2c2")
            ig2c = lc(ig2c_in, [GKP, 1], F32, "ig2c")
            fc1b = lc(fc1b_in, [HID, 1], F32, "fc1b")
            fc2b = lc(fc2b_in, [OUT, 1], F32, "fc2b")
            eyec = lc(eyec_in, [GKP, GKP], F32, "eyec")
            inveyec = lc(inveyec_in, [GKP, GKP], F32, "inveyec")

            stats = cpool.tile([128, 16], F32, tag="stats")
            nc.gpsimd.memset(stats[:], 0.0)
            xc10 = cpool.tile([GKP, HID], F32R, tag="xc10")
            DMA(xc10[:], z64_in[:])
            adjc = cpool.tile([GKP, GKP], F32, tag="adjc")
            nc.gpsimd.memset(adjc[:], 0.0)
            mz_all = cpool.tile([HID, G], F32R, tag="mz_all")

            def to_row(wt, nch, tag):
                """wt [128, nch] chunked column vector -> row tile [1, nch*128]
                in node order n = c*128 + p."""
                tp = psT.tile([nch, 128], F32R, tag="pTt")
                nc.tensor.transpose(tp[:], wt[:], eye[:])
                wtT = spool.tile([nch, 128], F32R, tag=f"{tag}T")
                nc.scalar.copy(wtT[:], tp[:])
                row = spool.tile([1, nch * 128], F32R, tag=f"{tag}R")
                a = wtT[:]
                dst = row[:]
                DMA(AP(dst.tensor, dst.offset, [[nch * 128, 1], [1, nch * 128]]),
                    AP(a.tensor, a.offset, [[128, nch], [1, 128]]))
                return row

            def bcast(row_ap, width, tag):
                """row_ap [1, width] -> [128, width] via gpsimd."""
                out = rpool.tile([128, width], row_ap.dtype, tag=tag)
                nc.gpsimd.partition_broadcast(out[:], row_ap, channels=128)
                return out

            def attention(hhat, es, q, grow, m_tiles, nheads, cdim):
                esr = kpool.tile(list(es.shape), F32R, tag="esr")
                nc.vector.tensor_copy(esr[:], es[:])
                """returns per head (np0, np1, rec_cols [128, NC])."""
                res = []
                for h in range(nheads):
                    dg = bcast(grow[h][0:1, :], N, "dg")
                    np0 = psA.tile([cdim, 512], F32, tag="pAa")
                    np1 = psA.tile([cdim, 512], F32, tag="pAb")
                    rp0 = psR.tile([1, 512], F32, tag="pRa")
                    rp1 = psR.tile([1, 512], F32, tag="pRb")
                    for j in range(NC):
                        # split i-halves so DVE and Pool mask-mult in parallel
                        zt = apool.tile([128, N], F32, tag="zt")
                        nc.vector.tensor_scalar(
                            zt[:, 0:512], dg[:, 0:512], q[:, j, h:h + 1], 1.0,
                            OP.mult, OP.max)
                        nc.vector.tensor_scalar(
                            zt[:, 512:N], dg[:, 512:N], q[:, j, h:h + 1], 1.0,
                            OP.mult, OP.max)
                        ft = fpool.tile([128, N], F32R, tag="ft")
                        nc.vector.tensor_tensor(
                            ft[:, 0:512], zt[:, 0:512], m_tiles[j][:, 0:512],
                            OP.mult)
                        nc.gpsimd.tensor_tensor(
                            ft[:, 512:N], zt[:, 512:N], m_tiles[j][:, 512:N],
                            OP.mult)
                        lhs = hhat[:, j, h * cdim:(h + 1) * cdim]
                        st, sp = (j == 0), (j == NC - 1)
                        nc.tensor.matmul(np0[:], lhs, ft[:, 0:512], start=st,
                                         stop=sp)
                        nc.tensor.matmul(rp0[:], esr[:, j, h:h + 1],
                                         ft[:, 0:512], start=st, stop=sp)
                        nc.tensor.matmul(np1[:], lhs, ft[:, 512:N], start=st,
                                         stop=sp)
                        nc.tensor.matmul(rp1[:], esr[:, j, h:h + 1],
                                         ft[:, 512:N], start=st, stop=sp)
                    rrow = spool.tile([1, N], F32, tag="rrow")
                    nc.scalar.copy(rrow[:, 0:512], rp0[:])
                    nc.vector.tensor_copy(rrow[:, 512:N], rp1[:])
                    rcp = psT.tile([128, NC], F32, tag="pTu")
                    for c in range(NC):
                        nc.tensor.transpose(rcp[:, c:c + 1],
                                            rrow[:, c * 128:(c + 1) * 128],
                                            eyef[0:1, 0:1])
                    rcol = spool.tile([128, NC], F32, tag="rcol")
                    nc.vector.tensor_copy(rcol[:], rcp[:])
                    rec = spool.tile([128, NC], F32, tag=f"rec{h}")
                    nc.vector.reciprocal(rec[:], rcol[:])
                    res.append((np0, np1, rec))
                return res

            for g in range(G):
                # ---------- adjacency ----------
                m_tiles = []
                for c in range(NC):
                    mt = mpool.tile([128, N], F32R, tag=f"m{c}")
                    DMA(mt[:], adj_in[g, c * 128:(c + 1) * 128, :])
                    m_tiles.append(mt)
                diag = spool.tile([128, NC], F32R, tag="diag")
                for c in range(NC):
                    src = AP(adj_in.tensor, g * N * N + c * 128 * (N + 1),
                             [[N + 1, 128], [1, 1]])
                    DMA(diag[:, c:c + 1], src)
                rsum = spool.tile([128, NC], F32, tag="rsum")
                for c in range(NC):
                    nc.vector.tensor_reduce(rsum[:, c:c + 1], m_tiles[c][:],
                                            AX.X, OP.add)
                    db = m_tiles[c][:, c * 128:(c + 1) * 128]
                    nc.vector.tensor_tensor(db, db, eye[:], OP.max)
                deg = spool.tile([128, NC], F32, tag="deg")
                nc.vector.scalar_tensor_tensor(deg[:], rsum[:], 1.0, diag[:],
                                               OP.add, OP.subtract)
                dln = spool.tile([128, NC], F32, tag="dln")
                nc.scalar.activation(dln[:], deg[:], F.Ln)
                dcol = spool.tile([128, NC], F32, tag="dcol")
                nc.scalar.activation(dcol[:], dln[:], F.Exp, scale=-0.5)
                nc.vector.tensor_reduce(stats[:, 0 + g:1 + g], rsum[:], AX.X,
                                        OP.add)

                # ---------- x load + transpose ----------
                xT = ppool.tile([F_IN, N], F32R, tag="xT")
                for c in range(NC):
                    xt = kpool.tile([128, F_IN], F32R, tag="xt")
                    DMA(xt[:], x_in[g, c * 128:(c + 1) * 128, :])
                    xp = psT.tile([F_IN, 128], F32R, tag="pTt")
                    nc.tensor.transpose(xp[:], xt[:], eye[:])
                    nc.scalar.copy(xT[:, c * 128:(c + 1) * 128], xp[:])

                # ---------- h_ext = x @ [W1|as|ad|p1W] ----------
                hhat1 = ppool.tile([128, NC, 2 * HID], F32R, tag="hhat1")
                es1 = ppool.tile([128, NC, H], F32, tag="es1")
                q1 = ppool.tile([128, NC, H], F32, tag="q1")
                dxwp = ppool.tile([128, NC, HID], F32R, tag="dxwp")
                for c in range(NC):
                    hp = psA.tile([128, 388], F32, tag="pAa")
                    nc.tensor.matmul(hp[:], xT[:, c * 128:(c + 1) * 128], w1[:],
                                     start=True, stop=True)
                    nc.scalar.activation(es1[:, c, :], hp[:, 256:258], F.Exp)
                    nc.scalar.activation(q1[:, c, :], hp[:, 256:258], F.Exp,
                                         scale=-0.8)
                    for h in range(H):
                        nc.vector.tensor_scalar(
                            hhat1[:, c, h * HID:(h + 1) * HID],
                            hp[:, h * HID:(h + 1) * HID], es1[:, c, h:h + 1],
                            None, OP.mult)
                    nc.vector.tensor_scalar(dxwp[:, c, :], hp[:, 260:388],
                                            dcol[:, c:c + 1], None, OP.mult)

                g1rows = []
                for h in range(H):
                    adp0 = psR.tile([1, 512], F32, tag="pRa")
                    adp1 = psR.tile([1, 512], F32, tag="pRb")
                    nc.tensor.matmul(adp0[:], w1[:, 258 + h:259 + h],
                                     xT[:, 0:512], start=True, stop=True)
                    nc.tensor.matmul(adp1[:], w1[:, 258 + h:259 + h],
                                     xT[:, 512:N], start=True, stop=True)
                    g1rowh = spool.tile([1, N], F32, tag=f"g1row{h}")
                    nc.scalar.activation(g1rowh[:, 0:512], adp0[:], F.Exp,
                                         scale=-0.8)
                    nc.scalar.activation(g1rowh[:, 512:N], adp1[:], F.Exp,
                                         scale=-0.8)
                    g1rows.append(g1rowh)

                # ---------- GAT1 ----------
                att1 = attention(hhat1, es1, q1, g1rows, m_tiles, H, HID)
                z1T = []
                for h in range(H):
                    np0, np1, rec = att1[h]
                    wt = spool.tile([128, NC], F32R, tag="wt")
                    nc.vector.tensor_tensor(wt[:], rec[:], ig1e[:], OP.mult)
                    wrow = to_row(wt, NC, "w1h")
                    wb = bcast(wrow[:], N, "wbr")
                    zT = ppool.tile([128, N], F32R, tag=f"z1T{h}")
                    for i2 in range(2):
                        sl = slice(i2 * 512, (i2 + 1) * 512)
                        npp = np0 if i2 == 0 else np1
                        u = kpool.tile([128, 512], F32, tag="u1")
                        nc.vector.scalar_tensor_tensor(
                            u[:], npp[:], 1.0, wb[:, sl], OP.mult, OP.mult)
                        v = kpool.tile([128, 512], F32, tag="v1")
                        eng = nc.gpsimd if i2 == 1 else nc.vector
                        eng.tensor_tensor(v[:], u[:],
                                          t2g1T[:, h, sl],
                                          OP.add)
                        nc.scalar.activation(zT[:, sl], v[:], F.Relu)
                    z1T.append(zT)

                # ---------- GAT2 ----------
                hhat2 = ppool.tile([128, NC, HID], F32R, tag="hhat2")
                es2 = ppool.tile([128, NC, 1], F32, tag="es2")
                q2 = ppool.tile([128, NC, 1], F32, tag="q2")
                for c in range(NC):
                    h2p = psA.tile([128, 256], F32, tag="pAb")
                    for h in range(H):
                        nc.tensor.matmul(h2p[:],
                                         z1T[h][:, c * 128:(c + 1) * 128],
                                         w2[:, h, :],
                                         start=(h == 0), stop=(h == 1))
                    nc.scalar.activation(es2[:, c, :], h2p[:, HID:HID + 1],
                                         F.Exp)
                    nc.scalar.activation(q2[:, c, :], h2p[:, HID:HID + 1],
                                         F.Exp, scale=-0.8)
                    nc.vector.tensor_scalar(hhat2[:, c, :], h2p[:, 0:HID],
                                            es2[:, c, 0:1], None, OP.mult)
                ad20 = psR.tile([1, 512], F32, tag="pRa")
                ad21 = psR.tile([1, 512], F32, tag="pRb")
                for h in range(H):
                    st, sp = (h == 0), (h == 1)
                    nc.tensor.matmul(ad20[:],
                                     w2[:, h, HID + 1:HID + 2],
                                     z1T[h][:, 0:512], start=st, stop=sp)
                    nc.tensor.matmul(ad21[:],
                                     w2[:, h, HID + 1:HID + 2],
                                     z1T[h][:, 512:N], start=st, stop=sp)
                g2row = spool.tile([1, N], F32, tag="g2row")
                nc.scalar.activation(g2row[:, 0:512], ad20[:], F.Exp, scale=-0.8)
                nc.scalar.activation(g2row[:, 512:N], ad21[:], F.Exp, scale=-0.8)

                att2 = attention(hhat2, es2, q2, [g2row], m_tiles, 1, HID)
                np0, np1, rec2 = att2[0]
                zl = ppool.tile([128, NC, HID], F32R, tag="zl")
                for c in range(NC):
                    npp = np0 if c < 4 else np1
                    off = (c % 4) * 128
                    tsb = kpool.tile([128, 128], F32R, tag="tsb")
                    nc.scalar.copy(tsb[:], npp[:, off:off + 128])
                    tp = psT.tile([128, 128], F32R, tag="pTt")
                    nc.tensor.transpose(tp[:], tsb[:], eye[:])
                    zraw = kpool.tile([128, HID], F32, tag="zraw")
                    nc.vector.scalar_tensor_tensor(
                        zraw[:], tp[:], rec2[:, c:c + 1],
                        g2bb[:], OP.mult, OP.add)
                    v2 = kpool.tile([128, HID], F32, tag="v2")
                    nc.vector.tensor_scalar(v2[:], zraw[:], 0.0, None, OP.max)
                    nc.scalar.activation(zl[:, c, :], v2[:], F.Relu,
                                         scale=ig2e[:, c:c + 1],
                                         bias=cb2e[:, c:c + 1])
                    DMA(zl_out[g, c * 128:(c + 1) * 128, :], zl[:, c, :])

                # ---------- pool branch GCN1 (flipped) ----------
                gp0 = psA.tile([HID, 512], F32, tag="pAa")
                gp1 = psA.tile([HID, 512], F32, tag="pAb")
                for j in range(NC):
                    st, sp = (j == 0), (j == NC - 1)
                    nc.tensor.matmul(gp0[:], dxwp[:, j, :], m_tiles[j][:, 0:512],
                                     start=st, stop=sp)
                    nc.tensor.matmul(gp1[:], dxwp[:, j, :], m_tiles[j][:, 512:N],
                                     start=st, stop=sp)
                wpt = spool.tile([128, NC], F32R, tag="wpt")
                nc.vector.tensor_tensor(wpt[:], dcol[:], ig1p[:], OP.mult)
                wprow = to_row(wpt, NC, "wp")
                wpb = bcast(wprow[:], N, "wbr")
                s1T = ppool.tile([HID, N], F32R, tag="s1T")
                for i2 in range(2):
                    sl = slice(i2 * 512, (i2 + 1) * 512)
                    npp = gp0 if i2 == 0 else gp1
                    u = kpool.tile([128, 512], F32, tag="u1")
                    nc.vector.scalar_tensor_tensor(
                        u[:], npp[:], 1.0, wpb[:, sl], OP.mult, OP.mult)
                    v = kpool.tile([128, 512], F32, tag="v1")
                    nc.gpsimd.tensor_tensor(v[:], u[:], t2p1T[:, sl], OP.add)
                    nc.scalar.activation(s1T[:, sl], v[:], F.Relu)

                # ---------- GCN2 ----------
                dsw = ppool.tile([128, NC, K6], F32R, tag="dsw")
                for c in range(NC):
                    swp = psT.tile([128, K6], F32, tag="pTt")
                    nc.tensor.matmul(swp[:], s1T[:, c * 128:(c + 1) * 128],
                                     p2w[:], start=True, stop=True)
                    nc.vector.tensor_scalar(dsw[:, c, :], swp[:],
                                            dcol[:, c:c + 1], None, OP.mult)
                sp0 = psA.tile([K6, 512], F32, tag="pAa")
                sp1 = psA.tile([K6, 512], F32, tag="pAb")
                for j in range(NC):
                    st, sp_ = (j == 0), (j == NC - 1)
                    nc.tensor.matmul(sp0[:], dsw[:, j, :], m_tiles[j][:, 0:512],
                                     start=st, stop=sp_)
                    nc.tensor.matmul(sp1[:], dsw[:, j, :], m_tiles[j][:, 512:N],
                                     start=st, stop=sp_)
                s2T = spool.tile([K6, N], F32R, tag="s2T")
                nc.scalar.copy(s2T[:, 0:512], sp0[:])
                nc.vector.tensor_copy(s2T[:, 512:N], sp1[:])
                wp2 = spool.tile([128, NC], F32, tag="wp2")
                nc.vector.tensor_tensor(wp2[:], dcol[:], ig2p[:], OP.mult)
                slog = ppool.tile([128, NC, K6], F32, tag="slog")
                for c in range(NC):
                    tpk = psT.tile([128, K6], F32R, tag="pTt")
                    nc.tensor.transpose(tpk[:], s2T[:, c * 128:(c + 1) * 128],
                                        eye[0:K6, 0:K6])
                    nc.vector.scalar_tensor_tensor(
                        slog[:, c, :], tpk[:], wp2[:, c:c + 1],
                        t2p2[:, c, :], OP.mult, OP.add)

                # ---------- double softmax over k ----------
                s_f = ppool.tile([128, NC, K6], F32R, tag="s_f")
                DMA(s_f[:], zk6_in[:])
                cur = slog[:, :, 0:K]
                for rep in range(2):
                    mx = kpool.tile([128, NC], F32, tag="mx")
                    nc.vector.tensor_reduce(mx[:], cur, AX.X, OP.max,
                                            negate=True)
                    ex = kpool.tile([128, NC, K], F32, tag="ex")
                    nc.vector.scalar_tensor_tensor(
                        ex[:], cur, 1.0, mx[:].broadcast_to([128, NC, K]),
                        OP.mult, OP.add)
                    ex2 = kpool.tile([128, NC, K], F32, tag="ex2")
                    nc.scalar.activation(ex2[:], ex[:], F.Exp)
                    sm = kpool.tile([128, NC], F32, tag="sm")
                    nc.vector.tensor_reduce(sm[:], ex2[:], AX.X, OP.add)
                    rc = kpool.tile([128, NC], F32, tag="rc")
                    nc.vector.reciprocal(rc[:], sm[:])
                    dst = kpool.tile([128, NC, K], F32, tag="smid")
                    nc.vector.scalar_tensor_tensor(
                        dst[:], ex2[:], 1.0, rc[:].broadcast_to([128, NC, K]),
                        OP.mult, OP.mult)
                    if rep == 1:
                        nc.vector.tensor_copy(s_f[:, :, 0:K], dst[:])
                    if rep == 0:
                        for c in range(NC):
                            DMA(s_out[g, c * 128:(c + 1) * 128, :], dst[:, c, :])
                    cur = dst[:]

                # ---------- ent loss ----------
                lg = kpool.tile([128, NC, K], F32, tag="lg")
                nc.scalar.activation(lg[:], s_f[:, :, 0:K], F.Ln,
                                     bias=epscol[:, 0:1])
                lg2 = kpool.tile([128, NC, K], F32, tag="lg2")
                entc = spool.tile([128, 1], F32, tag="entc")
                nc.vector.scalar_tensor_tensor(lg2[:], lg[:], 1.0,
                                               s_f[:, :, 0:K],
                                               OP.mult, OP.mult,
                                               accum_out=entc[:])
                nc.vector.tensor_scalar(stats[:, 2 + g:3 + g], entc[:], -1.0,
                                        None, OP.mult)

                # ---------- diffpool ----------
                ap0 = psA.tile([K6, 512], F32, tag="pAa")
                ap1 = psA.tile([K6, 512], F32, tag="pAb")
                for j in range(NC):
                    st, sp_ = (j == 0), (j == NC - 1)
                    nc.tensor.matmul(ap0[:], s_f[:, j, :],
                                     m_tiles[j][:, 0:512], start=st, stop=sp_)
                    nc.tensor.matmul(ap1[:], s_f[:, j, :],
                                     m_tiles[j][:, 512:N], start=st, stop=sp_)
                sTA = spool.tile([K6, N], F32R, tag="sTA")
                nc.scalar.copy(sTA[:, 0:512], ap0[:])
                nc.vector.tensor_copy(sTA[:, 512:N], ap1[:])
                vAs = ppool.tile([128, NC, K6], F32R, tag="vAs")
                for c in range(NC):
                    tpk = psT.tile([128, K6], F32R, tag="pTt")
                    nc.tensor.transpose(tpk[:], sTA[:, c * 128:(c + 1) * 128],
                                        eye[0:K6, 0:K6])
                    nc.vector.tensor_copy(vAs[:, c, :], tpk[:])
                wdg = spool.tile([128, NC], F32, tag="wdg")
                nc.vector.tensor_scalar(wdg[:], diag[:], -1.0, 1.0, OP.mult,
                                        OP.add)
                sw_ = ppool.tile([128, NC, K6], F32R, tag="sw_")
                for c in range(NC):
                    nc.vector.tensor_scalar(sw_[:, c, :], s_f[:, c, :],
                                            wdg[:, c:c + 1], None, OP.mult)
                oasb = spool.tile([K6, 2 * K6 + HID], F32, tag="oasb")
                corr = spool.tile([K6, K6], F32, tag="corr")
                for grp in range(4):
                    gp = psT.tile([K6, 2 * K6 + HID], F32, tag="pTt")
                    rng_ = [(0, K6), (K6, 2 * K6), (2 * K6, 2 * K6 + HID),
                            (0, K6)][grp]
                    for c in range(NC):
                        st, sp_ = (c == 0), (c == NC - 1)
                        lhs = sw_[:, c, :] if grp == 3 else s_f[:, c, :]
                        rhs = [vAs[:, c, :], s_f[:, c, :], zl[:, c, :],
                               s_f[:, c, :]][grp]
                        nc.tensor.matmul(gp[:, rng_[0]:rng_[1]], lhs, rhs,
                                         start=st, stop=sp_)
                    if grp == 3:
                        nc.scalar.copy(corr[:], gp[:, 0:K6])
                    else:
                        nc.scalar.copy(oasb[:, rng_[0]:rng_[1]],
                                       gp[:, rng_[0]:rng_[1]])
                oadj = spool.tile([K, K], F32, tag="oadj")
                nc.vector.tensor_tensor(oadj[:], oasb[0:K, 0:K],
                                        corr[0:K, 0:K], OP.subtract)
                tr2 = spool.tile([K, K], F32, tag="tr2")
                nc.vector.tensor_tensor(tr2[:], oadj[:], eyec[0:K, 0:K],
                                        OP.mult)
                nc.vector.tensor_reduce(stats[0:K, 4 + g:5 + g], tr2[:], AX.X,
                                        OP.add)
                stsq = spool.tile([K, K], F32, tag="stsq")
                nc.vector.tensor_tensor(stsq[:], oasb[0:K, K6:K6 + K],
                                        oasb[0:K, K6:K6 + K], OP.mult)
                nc.vector.tensor_reduce(stats[0:K, 6 + g:7 + g], stsq[:], AX.X,
                                        OP.add)
                nc.vector.tensor_copy(xc10[g * 32:g * 32 + K, :],
                                      oasb[0:K, 2 * K6:])
                nc.vector.tensor_copy(adjc[g * 32:g * 32 + K,
                                           g * 32:g * 32 + K], oadj[:])

                # ---------- out_local mean ----------
                mzp = psT.tile([HID, 2], F32, tag="pTu")
                for c in range(NC):
                    nc.tensor.matmul(mzp[:], zl[:, c, :], mzones[:],
                                     start=(c == 0), stop=(c == NC - 1))
                nc.scalar.copy(mz_all[:, g:g + 1], mzp[:, 0:1])

            # ================= coarse branch (both graphs, 10 rows) =========
            adjcsl = spool.tile([GKP, GKP], F32, tag="adjcsl")
            nc.vector.tensor_tensor(adjcsl[:], adjc[:], inveyec[:], OP.mult)
            nc.vector.tensor_tensor(adjcsl[:], adjcsl[:], eyec[:], OP.add)
            maskc = spool.tile([GKP, GKP], F32, tag="maskc")
            nc.vector.tensor_scalar(maskc[:], adjcsl[:], 0.0, None,
                                    OP.not_equal)

            def coarse_gat(xin_t, wsl, nheads, cdim, t2, igc, zname):
                nin = xin_t.shape[1]
                nf = nheads * cdim
                wcols = wsl[0].shape[-1]
                nchunks = (nin + 127) // 128
                xtp = []
                for h2 in range(nchunks):
                    w = min(128, nin - h2 * 128)
                    tp = psT.tile([128, GKP], F32R, tag="pTt")
                    nc.tensor.transpose(tp[0:w, :],
                                        xin_t[:, h2 * 128:h2 * 128 + w],
                                        eye[0:GKP, 0:GKP])
                    xt = spool.tile([128, GKP], F32R, tag=f"{zname}xT{h2}")
                    nc.scalar.copy(xt[0:w, :], tp[0:w, :])
                    xtp.append((xt, w))
                hcp = psT.tile([GKP, wcols], F32, tag="pTu")
                for i, (xt, w) in enumerate(xtp):
                    nc.tensor.matmul(hcp[:], xt[0:w, :], wsl[i],
                                     start=(i == 0), stop=(i == nchunks - 1))
                hc = spool.tile([GKP, wcols], F32R, tag=f"{zname}hc")
                nc.scalar.copy(hc[:], hcp[:])
                esc = spool.tile([GKP, nheads], F32, tag=f"{zname}es")
                nc.scalar.activation(esc[:], hc[:, nf:nf + nheads], F.Exp)
                qc = spool.tile([GKP, nheads], F32, tag=f"{zname}q")
                nc.scalar.activation(qc[:], hc[:, nf:nf + nheads], F.Exp,
                                     scale=-0.8)
                gcrs = []
                for h in range(nheads):
                    adps = psT.tile([1, GKP], F32, tag="pTt")
                    for i, (xt, w) in enumerate(xtp):
                        nc.tensor.matmul(
                            adps[:],
                            wsl[i][:, nf + nheads + h:nf + nheads + h + 1],
                            xt[0:w, :], start=(i == 0), stop=(i == nchunks - 1))
                    gcrh = spool.tile([1, GKP], F32, tag=f"{zname}gr{h}")
                    nc.scalar.activation(gcrh[:], adps[:], F.Exp, scale=-0.8)
                    gcrs.append(gcrh)
                zc = spool.tile([GKP, nf], F32R, tag=f"{zname}z")
                for h in range(nheads):
                    hhc = spool.tile([GKP, cdim], F32R, tag=f"{zname}hh")
                    nc.vector.tensor_scalar(hhc[:],
                                            hc[:, h * cdim:(h + 1) * cdim],
                                            esc[:, h:h + 1], None, OP.mult)
                    dgc = spool.tile([GKP, GKP], F32, tag=f"{zname}dg")
                    nc.gpsimd.partition_broadcast(dgc[:], gcrs[h][0:1, :],
                                                  channels=GKP)
                    zcc = spool.tile([GKP, GKP], F32, tag=f"{zname}zc")
                    nc.vector.tensor_scalar(zcc[:], dgc[:], qc[:, h:h + 1], 1.0,
                                            OP.mult, OP.max)
                    fcc = spool.tile([GKP, GKP], F32R, tag=f"{zname}fc")
                    nc.vector.tensor_tensor(fcc[:], zcc[:], maskc[:], OP.mult)
                    nump = psT.tile([cdim, GKP], F32, tag="pTt")
                    nc.tensor.matmul(nump[:], hhc[:], fcc[:], start=True,
                                     stop=True)
                    escr = spool.tile([GKP, 1], F32R, tag=f"{zname}esr")
                    nc.vector.tensor_copy(escr[:], esc[:, h:h + 1])
                    rpc = psT.tile([1, GKP], F32, tag="pTu")
                    nc.tensor.matmul(rpc[:], escr[:], fcc[:], start=True,
                                     stop=True)
                    nsb = spool.tile([cdim, GKP], F32R, tag=f"{zname}nsb")
                    nc.scalar.copy(nsb[:], nump[:])
                    rsb = spool.tile([1, GKP], F32, tag=f"{zname}rsb")
                    nc.scalar.copy(rsb[:], rpc[:])
                    ntp = psT.tile([GKP, cdim], F32R, tag="pTt")
                    nc.tensor.transpose(ntp[:], nsb[:], eye[:])
                    rtp = psT.tile([GKP, 1], F32, tag="pTu")
                    nc.tensor.transpose(rtp[:], rsb[:], eyef[0:1, 0:1])
                    rcc = spool.tile([GKP, 1], F32, tag=f"{zname}rcc")
                    nc.vector.tensor_copy(rcc[:], rtp[:])
                    recc = spool.tile([GKP, 1], F32, tag=f"{zname}recc")
                    nc.vector.reciprocal(recc[:], rcc[:])
                    wcc = spool.tile([GKP, 1], F32, tag=f"{zname}wcc")
                    nc.vector.tensor_tensor(wcc[:], recc[:], igc[:], OP.mult)
                    zpre = spool.tile([GKP, cdim], F32, tag=f"{zname}zpre")
                    nc.vector.scalar_tensor_tensor(
                        zpre[:], ntp[:], wcc[:, 0:1],
                        t2[:, h * cdim:(h + 1) * cdim], OP.mult, OP.add)
                    nc.scalar.activation(zc[:, h * cdim:(h + 1) * cdim],
                                         zpre[:], F.Relu)
                return zc

            z1c = coarse_gat(xc10, [wc1[:]], H, HID, t2c1, ig1c, "c1")
            zmeso = coarse_gat(z1c, [wc2[:, 0, :], wc2[:, 1, :]], 1, HID, t2c2, ig2c, "c2")
            for g in range(G):
                DMA(zm_out[g], zmeso[g * 32:g * 32 + K, :])

            # ---------- readout ----------
            mmp = psT.tile([HID, G], F32, tag="pTt")
            nc.tensor.matmul(mmp[:], zmeso[:], sel[:], start=True, stop=True)
            mmT = spool.tile([HID, G], F32R, tag="mmT")
            nc.scalar.copy(mmT[:], mmp[:])
            omp = psT.tile([HID, G], F32, tag="pTu")
            nc.tensor.matmul(omp[:], fc1[:], mmT[:], start=True, stop=True)
            omT = spool.tile([HID, G], F32R, tag="omT")
            nc.scalar.activation(omT[:], omp[:], F.Relu, bias=fc1b[:, 0:1])
            om2p = psT.tile([OUT, G], F32, tag="pTt")
            nc.tensor.matmul(om2p[:], fc2[:], omT[:], start=True, stop=True)
            omf = spool.tile([OUT, G], F32, tag="omf")
            nc.vector.tensor_scalar(omf[:], om2p[:], fc2b[:, 0:1], None, OP.add)
            olp = psT.tile([OUT, G], F32, tag="pTu")
            nc.tensor.matmul(olp[:], fc2[:], mz_all[:], start=True, stop=True)
            olf = spool.tile([OUT, G], F32, tag="olf")
            nc.vector.tensor_scalar(olf[:], olp[:], fc2b[:, 0:1], None, OP.add)
            aom = om_out[:]
            DMA(AP(aom.tensor, aom.offset, [[1, OUT], [OUT, G]]), omf[:])
            aol = ol_out[:]
            DMA(AP(aol.tensor, aol.offset, [[1, OUT], [OUT, G]]), olf[:])

            # ---------- stats ----------
            stp = psT.tile([16, 1], F32, tag="pTu")
            nc.tensor.matmul(stp[:], stats[:], ones[:], start=True, stop=True)
            stsb = spool.tile([16, 1], F32, tag="stsb")
            nc.scalar.copy(stsb[:], stp[:])
            DMA(st_out[:], stsb[:])

    nc.compile()
    _cache['nc'] = nc
    return nc


def _fold_params(p):
    d = {k: np.asarray(v, np.float64) for k, v in p.items()}

    def bnfold(pre):
        ig = d[pre + '_g'] / np.sqrt(d[pre + '_v'] + BN_EPS)
        c = d[pre + '_b'] - d[pre + '_m'] * ig
        return ig, c

    out = {}
    g1W = d['g1_W']
    was1 = np.stack([g1W.reshape(F_IN, H, HID)[:, h, :] @ d['g1_as'][h]
                     for h in range(H)], 1)
    wad1 = np.stack([g1W.reshape(F_IN, H, HID)[:, h, :] @ d['g1_ad'][h]
                     for h in range(H)], 1)
    out['wext1'] = np.concatenate([g1W, was1, wad1, d['p1_W']], 1)
    was2 = d['g2_W'] @ d['g2_as'][0]
    wad2 = d['g2_W'] @ d['g2_ad'][0]
    out['wext2'] = np.concatenate(
        [d['g2_W'], was2[:, None], wad2[:, None], np.zeros((2 * HID, 126))], 1)
    c1W = d['c1_W']
    wasc = np.stack([c1W.reshape(HID, H, HID)[:, h, :] @ d['c1_as'][h]
                     for h in range(H)], 1)
    wadc = np.stack([c1W.reshape(HID, H, HID)[:, h, :] @ d['c1_ad'][h]
                     for h in range(H)], 1)
    out['wc1ext'] = np.concatenate([c1W, wasc, wadc], 1)
    wasc2 = d['c2_W'] @ d['c2_as'][0]
    wadc2 = d['c2_W'] @ d['c2_ad'][0]
    out['wc2ext'] = np.concatenate(
        [d['c2_W'], wasc2[:, None], wadc2[:, None], np.zeros((2 * HID, 126))],
        1)
    out['p2w'] = np.concatenate([d['p2_W'], np.zeros((HID, 1))], 1)
    out['fc1'] = d['fc1_W']
    out['fc2'] = d['fc2_W']
    out['fc1b'] = d['fc1_b'][:, None]
    out['fc2b'] = d['fc2_b'][:, None]

    ig1e, c1e = bnfold('bn1e')
    ig2e, c2e = bnfold('bn2e')
    ig1p, c1p = bnfold('bn1p')
    ig2p, c2p = bnfold('bn2p')
    ig1c, c1c = bnfold('bn1c')
    ig2c, c2c = bnfold('bn2c')

    def cols(v):
        return np.ascontiguousarray(v.reshape(NC, 128).T)

    out['ig1e'] = cols(ig1e)
    out['ig2e'] = cols(ig2e)
    out['ig1p'] = cols(ig1p)
    out['ig2p'] = cols(ig2p)
    out['t2g1T'] = np.outer(d['g1_b'], ig1e) + c1e[None, :]
    out['g2bb'] = np.tile(d['g2_b'][None, :], (128, 1))
    out['cb2e'] = cols(c2e)
    out['t2p1T'] = np.outer(d['p1_b'], ig1p) + c1p[None, :]
    out['t2p2'] = np.concatenate([np.outer(ig2p, d['p2_b']) + c2p[:, None], np.zeros((N, 1))], 1)
    t2c1 = np.zeros((64, 2 * HID))
    t2c2 = np.zeros((64, HID))
    ig1cp = np.zeros((64, 1))
    ig2cp = np.zeros((64, 1))
    for g in range(G):
        t2c1[g * 32:g * 32 + K] = np.outer(ig1c, d['c1_b']) + c1c[:, None]
        t2c2[g * 32:g * 32 + K] = np.outer(ig2c, d['c2_b']) + c2c[:, None]
        ig1cp[g * 32:g * 32 + K, 0] = ig1c
        ig2cp[g * 32:g * 32 + K, 0] = ig2c
    out['t2c1'] = t2c1
    out['t2c2'] = t2c2
    out['ig1c'] = ig1cp
    out['ig2c'] = ig2cp

    out['eye128'] = np.eye(128)
    out['eye128f'] = np.eye(128)
    out['epscol'] = np.full((128, 1), 1e-15)
    out['zk6'] = np.zeros((128, NC, 6))
    out['zeros64'] = np.zeros((64, HID))
    out['eyec'] = np.eye(64)
    out['inveyec'] = 1.0 - np.eye(64)
    sel = np.zeros((64, G))
    for g in range(G):
        sel[g * 32:g * 32 + K, g] = 1.0 / K
    out['sel10'] = sel
    out['mzones'] = np.concatenate([np.full((128, 1), 1.0 / N), np.zeros((128, 1))], 1)
    out['ones128'] = np.ones((128, 1))
    return {k: np.ascontiguousarray(v, dtype=np.float32)
            for k, v in out.items()}


def kernel(x_dense, adj_dense, params):
    x = np.ascontiguousarray(np.asarray(x_dense), np.float32)
    adj = np.ascontiguousarray(np.asarray(adj_dense), np.float32)
    pf = _fold_params({k: np.asarray(v) for k, v in params.items()})
    nc = build_program()
    in_maps = []
    for core in range(NCORES):
        m = dict(pf)
        m['x'] = x[core * G:(core + 1) * G]
        m['adj'] = adj[core * G:(core + 1) * G]
        in_maps.append(m)
    res = run_bass_kernel_spmd(nc, in_maps, core_ids=list(range(NCORES)))
    z_local = np.concatenate([r['z_local'] for r in res.results], 0)
    z_meso = np.concatenate([r['z_meso'] for r in res.results], 0)
    s = np.concatenate([r['s'] for r in res.results], 0)
    out_local = np.concatenate([r['out_local'] for r in res.results], 0)
    out_meso = np.concatenate([r['out_meso'] for r in res.results], 0)
    link_sq = 0.0
    ent_sum = 0.0
    for r in res.results:
        st = r['stats'][:, 0].astype(np.float64)
        for g in range(G):
            link_sq += st[0 + g] - 2.0 * st[4 + g] + st[6 + g]
            ent_sum += st[2 + g]
    link_loss = np.float32(np.sqrt(link_sq) / (B * N * N))
    ent_loss = np.float32(ent_sum / (B * N))
    return (z_local, z_meso, s, out_local, out_meso, link_loss, ent_loss)
